# revision 1
# baseline (speedup 1.0000x reference)
"""Trainium2 Bass kernel for nn_Actor_56916906607124 (compute_encoder_mask).

Computation (per batch instance b, row i):
  mask[b,i,j] = 1 iff  (j is among the 16 nearest time-window-compatible,
                        non-diagonal neighbors of i)  OR depot[b,i]  OR
                        depot[b,j]  OR i == j.

Sharding: pure data parallelism — batch B=8 across 8 NeuronCores, one
instance per core.  No collectives.

Per-core algorithm (N=2048, 16 row-tiles of 128 rows):
  selection key  x = twc - d  (eligible j: x in (0,1]; blocked j: x <= 0);
  diagonal forced out of selection (x -= 2.5 on the diag block).
  16th-largest x per row (DVE): 4 chunk-wise `max` (top-8 of each 512-col
  chunk) -> 32 candidates, global `max`, `match_replace` removes those 8,
  `max` again -> v8b[:,7] = 16th largest.  A chunk holding >8 of the true
  top-16 under-estimates the threshold, which the count check catches.
  knn = (x >= t16) in bf16 with a per-row count (accum_out) shipped out.
  The ORs run on the otherwise-idle engines: the TensorEngine accumulates
  knn + depot_col + diag-identity into PSUM via identity matmuls (sums in
  {0..3}, exact), and the Scalar engine's Sign LUT with a per-partition
  bias of 2*depot_row emits the final 0/1 mask directly as uint8 (the host
  widens to f32 while unsharding -- the store shrinks 4x).
  Mask stores are issued on the sync HWDGE queue after every load so loads
  stream at pure rate; the first tiles load in column pieces to start the
  DVE early.
  Host repairs rows whose count != 16 or whose raw threshold is <= 0
  (ignoring depot rows, which are all-ones by construction) by exact numpy
  reference recomputation -- float ties at the 8/9 or 16/17 selection
  boundary, chunk mis-coverage, or <16 eligible neighbors.  O(N) per
  flagged row; ~450 of 16384 rows on the seed-0 data, dominated by the
  4-chunk coverage heuristic, all repaired exactly.
"""

from contextlib import ExitStack

import numpy as np

import concourse.bass as bass
import concourse.mybir as mybir
from concourse import bacc, tile

B, N, P = 8, 2048, 128
NT = N // P  # 16 row-tiles per core
K = 16
f32 = mybir.dt.float32
i32 = mybir.dt.int32
bf16 = mybir.dt.bfloat16
Alu = mybir.AluOpType
Act = mybir.ActivationFunctionType

_program_cache = {}


def build_program():
    if "nc" in _program_cache:
        return _program_cache["nc"]
    nc = bacc.Bacc()
    d_h = nc.declare_dram_parameter("d", [N, N], f32, isOutput=False)
    twc_h = nc.declare_dram_parameter("twc", [N, N], i32, isOutput=False)
    dflat_h = nc.declare_dram_parameter("dflat", [1, N], bf16, isOutput=False)
    drow2_h = nc.declare_dram_parameter("drow2", [P, NT], f32, isOutput=False)
    ident_h = nc.declare_dram_parameter("ident", [P, P], bf16, isOutput=False)
    mask_h = nc.declare_dram_parameter("mask", [N, N], mybir.dt.uint8,
                                       isOutput=True)
    nge_h = nc.declare_dram_parameter("nge", [P, NT], f32, isOutput=True)
    t16r_h = nc.declare_dram_parameter("t16r", [P, NT], f32, isOutput=True)

    with ExitStack() as ctx:
        tc = ctx.enter_context(tile.TileContext(nc))
        const = ctx.enter_context(tc.tile_pool(name="const", bufs=1))
        inp = ctx.enter_context(tc.tile_pool(name="inp", bufs=5))
        work = ctx.enter_context(tc.tile_pool(name="work", bufs=4))
        outp = ctx.enter_context(tc.tile_pool(name="outp", bufs=16))
        small = ctx.enter_context(tc.tile_pool(name="small", bufs=6))
        psum = ctx.enter_context(
            tc.tile_pool(name="psum", bufs=4, space="PSUM"))

        dflat_s = const.tile([1, N], bf16)
        ones_s = const.tile([1, P], bf16)
        dc01_s = const.tile([P, N], bf16)
        drow2_s = const.tile([P, NT], f32)
        ident_s = const.tile([P, P], bf16)
        nge_s = const.tile([P, NT], f32)
        v8ball = const.tile([P, 8 * NT], f32)

        def load_consts():
            # issued AFTER tile-0's loads: every small DMA costs ~650 ns of
            # SP dispatch, and putting them first delays the first compute
            # data by ~2.7 us.  Nothing needs these before ~8 us.
            nc.sync.dma_start(dflat_s[:], dflat_h[:, :])
            nc.gpsimd.memset(ones_s[:], 1.0)
            # dc01 (depot broadcast across partitions) on-chip: K=1 matmul
            # replicates the depot row; the Scalar engine narrows to bf16
            for c in range(4):
                cols = slice(c * 512, (c + 1) * 512)
                pt = psum.tile([P, 512], f32, tag="pb")
                nc.tensor.matmul(pt[:], ones_s[:], dflat_s[:, cols])
                nc.scalar.activation(dc01_s[:, cols], pt[:], Act.Copy)
            nc.sync.dma_start(drow2_s[:], drow2_h[:, :])
            nc.sync.dma_start(ident_s[:], ident_h[:, :])

        NCH = 4          # selection chunks per row
        CW = N // NCH    # 512 columns per chunk
        pending_stores = []
        for r in range(NT):
            rows = slice(r * P, (r + 1) * P)
            d_t = inp.tile([P, N], f32, tag="d")
            twc_t = inp.tile([P, N], i32, tag="twc")
            x = work.tile([P, N], f32, tag="x")
            if r <= 1:
                # first two tiles: load + build x in column pieces so the
                # DVE starts right after the first half-megabyte and is not
                # starved while the load stream ramps
                pieces = ([(0, 512), (512, 512), (1024, 1024)] if r == 0
                          else [(0, 1024), (1024, 1024)])
                for (c0, w) in pieces:
                    cs = slice(c0, c0 + w)
                    nc.sync.dma_start(d_t[:, cs], d_h[rows, cs])
                    nc.sync.dma_start(twc_t[:, cs], twc_h[rows, cs])
                    nc.vector.tensor_tensor(
                        x[:, cs], twc_t[:, cs], d_t[:, cs], Alu.subtract)
                if r == 0:
                    load_consts()
            else:
                nc.sync.dma_start(d_t[:], d_h[rows, :])
                nc.sync.dma_start(twc_t[:], twc_h[rows, :])
                # x = twc - d: eligible j have x in (0,1], blocked j have
                # x <= 0, so the top-16 of x = the 16 nearest eligible
                nc.vector.tensor_tensor(x[:], twc_t[:], d_t[:], Alu.subtract)
            # exclude diagonal from selection: x_diag -= 2.5
            xblk = x[:, rows]
            nc.vector.scalar_tensor_tensor(
                xblk, ident_s[:], -2.5, xblk, Alu.mult, Alu.add
            )
            # per-chunk top-8 -> 64 candidates.  The true top-16 is contained
            # in the candidates unless one 256-chunk holds >8 of it; that rare
            # case makes the computed threshold strictly smaller, so the row
            # count comes out > 16 and the host repairs the row exactly.
            cand = small.tile([P, NCH * 8], f32, tag="cand")
            for c in range(NCH):
                nc.vector.max(cand[:, c * 8 : (c + 1) * 8],
                              x[:, c * CW : (c + 1) * CW])
            # global top-8 (always exact: a chunk top-8 covers its share)
            v8a = small.tile([P, 8], f32, tag="v8a")
            nc.vector.max(v8a[:], cand[:])
            # remove exactly those 8 from the candidates, then next-8
            cand2 = small.tile([P, NCH * 8], f32, tag="cand2")
            nc.vector.match_replace(cand2[:], v8a[:], cand[:], -1e30)
            v8b = v8ball[:, r * 8 : (r + 1) * 8]
            nc.vector.max(v8b, cand2[:])
            # knn = (x >= 16th largest), nge[:, r] = per-row count
            # (diag still excluded: x_diag <= -0.5 < t16, so the count is a
            #  pure top-16 count -- 16 unless a float tie at a boundary)
            # t16' = 16th largest - 1e30*depot_row: depot rows compare all-true
            # (whole row is 1 in the reference), and the host ignores their
            # count when flagging tie rows.
            knn = work.tile([P, N], bf16, tag="knn")
            if r == NT - 1:
                # last tile: compare in halves so the PE/ACT/store drain
                # starts while the second half is still comparing
                na = small.tile([P, 1], f32, tag="na")
                nb = small.tile([P, 1], f32, tag="nb")
                half = N // 2
                nc.vector.tensor_scalar(
                    knn[:, :half], x[:, :half], v8ball[:, r * 8 + 7 : r * 8 + 8], None,
                    Alu.is_ge, Alu.add, accum_out=na[:])
                nc.vector.tensor_scalar(
                    knn[:, half:], x[:, half:], v8ball[:, r * 8 + 7 : r * 8 + 8], None,
                    Alu.is_ge, Alu.add, accum_out=nb[:])
                nc.vector.tensor_tensor(
                    nge_s[:, r : r + 1], na[:], nb[:], Alu.add)
            else:
                nc.vector.tensor_scalar(
                    knn[:], x[:], v8ball[:, r * 8 + 7 : r * 8 + 8], None, Alu.is_ge, Alu.add,
                    accum_out=nge_s[:, r : r + 1],
                )
            # out = knn OR depot_col on the idle engines: identity-matmul
            # accumulate knn + dc01 into PSUM (sums in {0,1,2}, exact), then
            # the Scalar engine's Sign LUT emits the 0/1 mask as uint8
            # (host widens to f32) -- the store shrinks 4x and the DVE is
            # freed of the OR pass entirely.
            # force the diagonal on (after the count accum; tiny bf16 2x op)
            kblk = knn[:, rows]
            nc.vector.tensor_tensor(kblk, kblk, ident_s[:], Alu.logical_or)
            out_t = outp.tile([P, N], mybir.dt.uint8, tag="out")
            for c in range(4):
                cols = slice(c * 512, (c + 1) * 512)
                pt = psum.tile([P, 512], f32, tag="acc")
                nc.tensor.matmul(pt[:], ident_s[:], knn[:, cols],
                                 start=True, stop=False)
                nc.tensor.matmul(pt[:], ident_s[:], dc01_s[:, cols],
                                 start=False, stop=True)
                nc.scalar.activation(out_t[:, cols], pt[:], Act.Sign,
                                     bias=drow2_s[:, r : r + 1])
            pending_stores.append((rows, out_t))

        # all mask stores issued on the sync queue AFTER every load: the
        # HWDGE FIFO then gives loads strict priority, so the load stream
        # runs at pure rate and compute never starves; stores fill the
        # DMA idle time at the end of the stream.
        for rows, out_t in pending_stores[:-1]:
            nc.sync.dma_start(mask_h[rows, :], out_t[:])
        # the final tile's store rides the otherwise-idle scalar queue so it
        # needn't wait behind the 15 queued sync-stores at the drain
        rows, out_t = pending_stores[-1]
        nc.scalar.dma_start(mask_h[rows, :], out_t[:])
        nc.scalar.dma_start(nge_h[:, :], nge_s[:])
        t16r_view = v8ball[:].rearrange("p (nt e) -> p nt e", e=8)[:, :, 7]
        nc.scalar.dma_start(t16r_h[:, :], t16r_view)

    nc.compile()
    _program_cache["nc"] = nc
    return nc


def _repair_row(d_row, twc_row, depot_b, max_dist_b, i):
    """Exact float32 re-computation of reference row i (handles ties)."""
    n = d_row.shape[0]
    m = (twc_row == 0).astype(np.float32)
    m[i] = np.float32(1.0)
    big = (m * np.float32(max_dist_b)) * np.float32(10.0)
    dist = d_row * (np.float32(1.0) - m) + big
    idx = np.argsort(dist, kind="stable")[:K]
    knn = np.zeros(n, np.float32)
    knn[idx] = 1.0
    knn *= (twc_row == 1)
    dep = (depot_b + depot_b[i]) > 0
    out = ((knn > 0) | dep | (np.arange(n) == i)).astype(np.float32)
    return out


def make_in_maps(distance_matrix, time_window_compatibility, depot):
    bf = mybir.dt.np(bf16)
    ident = np.eye(P, dtype=bf)
    in_maps = []
    for b in range(B):
        dep_f = depot[b].astype(np.float32)
        in_maps.append({
            "d": distance_matrix[b],
            "twc": time_window_compatibility[b],
            "dflat": np.ascontiguousarray(dep_f.astype(bf).reshape(1, N)),
            "drow2": np.ascontiguousarray(
                (dep_f * np.float32(2.0)).reshape(NT, P).T),
            "ident": ident,
        })
    return in_maps


def _get_executor():
    """Build the 8-core shard_map executable once (mirrors
    bass2jax.run_bass_via_pjrt, but cached so repeat calls skip retracing)."""
    if "exec" in _program_cache:
        return _program_cache["exec"]
    import jax
    from jax.sharding import Mesh, NamedSharding, PartitionSpec
    from jax.experimental.shard_map import shard_map
    from concourse import bass2jax
    from concourse.bass2jax import _bass_exec_p, install_neuronx_cc_hook

    nc = build_program()
    install_neuronx_cc_hook()
    partition_name = (nc.partition_id_tensor.name
                      if nc.partition_id_tensor else None)
    in_names, out_names, out_avals = [], [], []
    for alloc in nc.m.functions[0].allocations:
        if not isinstance(alloc, mybir.MemoryLocationSet):
            continue
        name = alloc.memorylocations[0].name
        if alloc.kind == "ExternalInput":
            if name != partition_name:
                in_names.append(name)
        elif alloc.kind == "ExternalOutput":
            out_names.append(name)
            out_avals.append(jax.core.ShapedArray(
                tuple(alloc.tensor_shape), mybir.dt.np(alloc.dtype)))
    all_in_names = list(in_names) + list(out_names)
    if partition_name is not None:
        all_in_names.append(partition_name)

    def _body(*args):
        operands = list(args)
        if partition_name is not None:
            operands.append(bass2jax.partition_id_tensor())
        return tuple(_bass_exec_p.bind(
            *operands,
            out_avals=tuple(out_avals),
            in_names=tuple(all_in_names),
            out_names=tuple(out_names),
            lowering_input_output_aliases=(),
            sim_require_finite=True,
            sim_require_nnan=True,
            nc=nc,
        ))

    devices = jax.devices()[:B]
    mesh = Mesh(np.asarray(devices), ("core",))
    spec = PartitionSpec("core")
    n_io = len(in_names) + len(out_names)
    sharded = jax.jit(
        shard_map(_body, mesh=mesh, in_specs=(spec,) * n_io,
                  out_specs=(spec,) * len(out_names), check_rep=False),
        donate_argnums=tuple(range(len(in_names), n_io)), keep_unused=True,
    )
    sharding = NamedSharding(mesh, spec)
    ex = (sharded, in_names, out_names, out_avals, sharding)
    _program_cache["exec"] = ex
    return ex


def _run_device(args_dev):
    import jax

    sharded, in_names, out_names, out_avals, sharding = _get_executor()
    # the kernel fully overwrites all outputs; donate last call's buffers
    prev = _program_cache.get("outs")
    if prev is None:
        prev = tuple(jax.device_put(
            np.zeros((B * av.shape[0], *av.shape[1:]), av.dtype), sharding)
            for av in out_avals)
    outs_dev = sharded(*args_dev, *prev)
    _program_cache["outs"] = outs_dev
    return {n: np.array(a).reshape(B, *out_avals[i].shape)
            for i, (n, a) in enumerate(zip(out_names, outs_dev))}


def kernel(distance_matrix, max_dist, time_window_compatibility, depot,
           num_neighbors_encoder):
    import jax

    distance_matrix = np.asarray(distance_matrix, dtype=np.float32)
    time_window_compatibility = np.asarray(time_window_compatibility,
                                           dtype=np.int32)
    depot = np.asarray(depot, dtype=np.int32)
    max_dist = np.asarray(max_dist, dtype=np.float32).reshape(B)
    assert int(np.asarray(num_neighbors_encoder)) == K
    assert distance_matrix.shape == (B, N, N)

    sharded, in_names, out_names, out_avals, sharding = _get_executor()
    in_maps = make_in_maps(distance_matrix, time_window_compatibility, depot)
    concat_in = [np.concatenate([in_maps[c][n] for c in range(B)], axis=0)
                 for n in in_names]
    args_dev = [jax.device_put(a, sharding) for a in concat_in]

    rng = np.random.default_rng(0)
    for attempt in range(3):
        by_name = _run_device(args_dev)
        out = by_name["mask"].astype(np.float32)  # widen device's 0/1 uint8
        nge = by_name["nge"]      # [B, P, NT]
        t16r = by_name["t16r"]

        # exact repair of rows with a float tie at a selection boundary, or
        # with fewer than 16 eligible neighbors (t16r <= 0).  Depot rows are
        # all-ones by construction and never need repair.
        flag = ((nge != np.float32(K)) | (t16r <= 0)) & (
            depot.reshape(B, NT, P).transpose(0, 2, 1) == 0)
        for b, p, r in zip(*np.nonzero(flag)):
            i = int(r) * P + int(p)
            out[b, i] = _repair_row(
                distance_matrix[b, i], time_window_compatibility[b, i],
                depot[b], max_dist[b], i,
            )

        # audit: recompute a random sample of rows exactly on host; any
        # mismatch indicates a transient device glitch -> rerun the call
        ok = True
        for _ in range(192):
            b = int(rng.integers(B))
            i = int(rng.integers(N))
            exp = _repair_row(
                distance_matrix[b, i], time_window_compatibility[b, i],
                depot[b], max_dist[b], i,
            )
            if not np.array_equal(out[b, i], exp):
                ok = False
                break
        if ok:
            return out
    return out



# revision 4
# speedup vs baseline: 3.4359x; 3.4359x over previous
"""Trainium2 Bass kernel for nn_Actor_56916906607124 (compute_encoder_mask).

Computation (per batch instance b, row i):
  mask[b,i,j] = 1 iff  (j is among the 16 nearest time-window-compatible,
                        non-diagonal neighbors of i)  OR depot[b,i]  OR
                        depot[b,j]  OR i == j.

Sharding: pure data parallelism -- batch B=8 across 8 NeuronCores, one
instance per core.  No collectives.

Key structural facts exploited:
  * depot rows are all-ones and depot columns are all-ones in the output,
    independent of the KNN result.  Only non-depot rows (~1024 of 2048 per
    instance) need the device; the host memsets the rest while unsharding.
  * the selection key x = (twc && !diag) ? -d : -3 folds both inputs into a
    single bf16 tensor: eligible j have x = -d in (-1, 0], blocked j sit at
    -3, and the 16 nearest eligible neighbors are exactly the top-16 of x.
    bf16 rounding is monotone, so the bf16 top-16 equals the f32 top-16
    unless two values collide at the 16/17 boundary -- which the count
    check flags for exact host repair.

Per-core device program (R=1152 padded non-depot rows, 9 tiles of 128):
  DMA   : x tile [128,2048] bf16 in (4096 B/row descriptors, full rate).
  DVE   : folded = max(x[:, :1024], x[:, 1024:])  (bf16 2x mode; the Pool
          engine cannot run ALU ops on core v3);
          4x max8 over 256-wide chunks of folded -> 32 candidates;
          max8 -> top-8, match_replace, max8 -> ranks 9..16 => t16;
          bias = -t16 + eps;  is_ge count over the 896 non-stored cols
          (4x DVE mode: all-bf16 packed operands).
  ACT   : Sign(x + bias) SBUF->SBUF straight to uint8 over the 1152 stored
          cols (negatives wrap to 255; host maps ==1) with the accumulator
          shipping #sel - #unsel per row.
  DMA   : mask tile [128,1152] uint8 out on the scalar queue.

Host flags rows with count != 16 (boundary tie in bf16, fold collision, or
chunk-coverage miss -- any wrong t16 shifts the count off 16), t16 <= -2
(fewer than 16 eligible) or |t16| < 1e-3 (eps-guard margin), and recomputes
exactly those rows in f32 numpy.  ~950 of ~8100 rows on the seed-0 data;
verified to cover every differing row.
"""

from contextlib import ExitStack

import numpy as np

import concourse.bass as bass
import concourse.mybir as mybir
from concourse import bacc, tile

B, N, P = 8, 2048, 128
K = 16
EPS = 1e-7
f32 = mybir.dt.float32
bf16 = mybir.dt.bfloat16
u8 = mybir.dt.uint8
Alu = mybir.AluOpType
Act = mybir.ActivationFunctionType

_program_cache = {}


def build_program(rt=9):
    """Device program for RT row-tiles of 128 non-depot rows."""
    key = ("nc", rt)
    if key in _program_cache:
        return _program_cache[key]
    R = rt * P          # padded non-depot rows
    C = min(R, N)       # stored (non-depot-first) columns
    REST = N - C        # trailing depot columns: counted, not stored

    nc = bacc.Bacc()
    x_h = nc.declare_dram_parameter("x", [R, N], bf16, isOutput=False)
    mask_h = nc.declare_dram_parameter("mask", [R, C], u8, isOutput=True)
    cnt_h = nc.declare_dram_parameter("cnt", [P, rt], f32, isOutput=True)
    acc_h = nc.declare_dram_parameter("acc", [P, rt], f32, isOutput=True)
    t16_h = nc.declare_dram_parameter("t16", [P, rt], f32, isOutput=True)

    H = N // 2
    with ExitStack() as ctx:
        tc = ctx.enter_context(tile.TileContext(nc))
        const = ctx.enter_context(tc.tile_pool(name="const", bufs=1))
        inp = ctx.enter_context(tc.tile_pool(name="inp", bufs=4))
        fold = ctx.enter_context(tc.tile_pool(name="fold", bufs=3))
        outp = ctx.enter_context(tc.tile_pool(name="outp", bufs=3))
        small = ctx.enter_context(tc.tile_pool(name="small", bufs=4))

        v8ball = const.tile([P, 8 * rt], f32)
        bias_s = const.tile([P, rt], f32)
        cnt_s = const.tile([P, rt], f32)
        acc_s = const.tile([P, rt], f32)

        for r in range(rt):
            rows = slice(r * P, (r + 1) * P)
            x_t = inp.tile([P, N], bf16, tag="x")
            f_t = fold.tile([P, H], bf16, tag="f")
            if r == 0:
                # ramp: load in column pieces so the fold starts after half
                # the tile, and fold in halves so the DVE starts earlier
                nc.sync.dma_start(x_t[:, 0:512], x_h[rows, 0:512])
                nc.sync.dma_start(x_t[:, H : H + 512], x_h[rows, H : H + 512])
                nc.sync.dma_start(x_t[:, 512:H], x_h[rows, 512:H])
                nc.sync.dma_start(x_t[:, H + 512 :], x_h[rows, H + 512 :])
                nc.vector.tensor_tensor(
                    f_t[:, 0:512], x_t[:, 0:512], x_t[:, H : H + 512], Alu.max)
                nc.vector.tensor_tensor(
                    f_t[:, 512:], x_t[:, 512:H], x_t[:, H + 512 :], Alu.max)
            else:
                nc.sync.dma_start(x_t[:], x_h[rows, :])
                # folded[j] = max(x[j], x[j+1024]): any top-16 member of x
                # survives into folded unless its partner also is one (fold
                # collision) -- then t16 comes out low and the count flags.
                nc.vector.tensor_tensor(
                    f_t[:], x_t[:, :H], x_t[:, H:], Alu.max)
            # per-chunk top-8 of folded -> 32 candidates
            cand = small.tile([P, 32], f32, tag="cand")
            for c in range(4):
                nc.vector.max(cand[:, c * 8 : (c + 1) * 8],
                              f_t[:, c * 256 : (c + 1) * 256])
            v8a = small.tile([P, 8], f32, tag="v8a")
            nc.vector.max(v8a[:], cand[:])
            cand2 = small.tile([P, 32], f32, tag="cand2")
            nc.vector.match_replace(cand2[:], v8a[:], cand[:], -1e30)
            v8b = v8ball[:, r * 8 : (r + 1) * 8]
            nc.vector.max(v8b, cand2[:])
            t16 = v8ball[:, r * 8 + 7 : r * 8 + 8]
            # ACT bias: -t16 + EPS (EPS < any bf16 gap at |t16| >= 1e-3, so
            # Sign(x + bias) > 0  <=>  x >= t16; |t16| < 1e-3 rows flagged)
            nc.vector.tensor_scalar(
                bias_s[:, r : r + 1], t16, -1.0, EPS, Alu.mult, Alu.add)
            if REST:
                # top-16 membership count over the non-stored (depot) cols;
                # in-place over x so no extra SBUF, 4x DVE mode (all bf16)
                nc.vector.tensor_scalar(
                    x_t[:, C:], x_t[:, C:], t16, None, Alu.is_ge, Alu.add,
                    accum_out=cnt_s[:, r : r + 1])
            else:
                nc.vector.memset(cnt_s[:, r : r + 1], 0.0)
            # stored mask: Sign gives 1 / 0 / -1(->255 as uint8); the
            # accumulator ships  #sel - #unsel  so count = (acc + C) / 2
            out_t = outp.tile([P, C], u8, tag="out")
            nc.scalar.activation(out_t[:], x_t[:, :C], Act.Sign,
                                 bias=bias_s[:, r : r + 1],
                                 accum_out=acc_s[:, r : r + 1])
            nc.scalar.dma_start(mask_h[rows, :], out_t[:])

        nc.scalar.dma_start(cnt_h[:, :], cnt_s[:])
        nc.scalar.dma_start(acc_h[:, :], acc_s[:])
        t16v = v8ball[:].rearrange("p (nt e) -> p nt e", e=8)[:, :, 7]
        nc.scalar.dma_start(t16_h[:, :], t16v)

    nc.compile()
    _program_cache[key] = nc
    return nc


def _repair_row(d_row, twc_row, depot_b, max_dist_b, i):
    """Exact float32 re-computation of reference row i (handles ties)."""
    n = d_row.shape[0]
    m = (twc_row == 0).astype(np.float32)
    m[i] = np.float32(1.0)
    big = (m * np.float32(max_dist_b)) * np.float32(10.0)
    dist = d_row * (np.float32(1.0) - m) + big
    idx = np.argsort(dist, kind="stable")[:K]
    knn = np.zeros(n, np.float32)
    knn[idx] = 1.0
    knn *= (twc_row == 1)
    dep = (depot_b + depot_b[i]) > 0
    out = ((knn > 0) | dep | (np.arange(n) == i)).astype(np.float32)
    return out


def _prep_core(d_b, twc_b, depot_b, rt, not_eye):
    """Build the per-core compacted selection-key tensor + index maps."""
    R = rt * P
    bf = mybir.dt.np(bf16)
    nd = np.flatnonzero(depot_b == 0)
    dep = np.flatnonzero(depot_b == 1)
    colperm = np.concatenate([nd, dep])
    xf = np.where((twc_b == 1) & not_eye, -d_b, np.float32(-3.0))
    xc = np.full((R, N), np.float32(-3.0), np.float32)
    xc[: len(nd)] = xf[nd][:, colperm]
    return xc.astype(bf), nd, colperm


def _get_executor(rt=9):
    """Build the 8-core shard_map executable once (mirrors
    bass2jax.run_bass_via_pjrt, but cached so repeat calls skip retracing)."""
    key = ("exec", rt)
    if key in _program_cache:
        return _program_cache[key]
    import jax
    from jax.sharding import Mesh, NamedSharding, PartitionSpec
    from jax.experimental.shard_map import shard_map
    from concourse import bass2jax
    from concourse.bass2jax import _bass_exec_p, install_neuronx_cc_hook

    nc = build_program(rt)
    install_neuronx_cc_hook()
    partition_name = (nc.partition_id_tensor.name
                      if nc.partition_id_tensor else None)
    in_names, out_names, out_avals = [], [], []
    for alloc in nc.m.functions[0].allocations:
        if not isinstance(alloc, mybir.MemoryLocationSet):
            continue
        name = alloc.memorylocations[0].name
        if alloc.kind == "ExternalInput":
            if name != partition_name:
                in_names.append(name)
        elif alloc.kind == "ExternalOutput":
            out_names.append(name)
            out_avals.append(jax.core.ShapedArray(
                tuple(alloc.tensor_shape), mybir.dt.np(alloc.dtype)))
    all_in_names = list(in_names) + list(out_names)
    if partition_name is not None:
        all_in_names.append(partition_name)

    def _body(*args):
        operands = list(args)
        if partition_name is not None:
            operands.append(bass2jax.partition_id_tensor())
        return tuple(_bass_exec_p.bind(
            *operands,
            out_avals=tuple(out_avals),
            in_names=tuple(all_in_names),
            out_names=tuple(out_names),
            lowering_input_output_aliases=(),
            sim_require_finite=True,
            sim_require_nnan=True,
            nc=nc,
        ))

    devices = jax.devices()[:B]
    mesh = Mesh(np.asarray(devices), ("core",))
    spec = PartitionSpec("core")
    n_io = len(in_names) + len(out_names)
    sharded = jax.jit(
        shard_map(_body, mesh=mesh, in_specs=(spec,) * n_io,
                  out_specs=(spec,) * len(out_names), check_rep=False),
        donate_argnums=tuple(range(len(in_names), n_io)), keep_unused=True,
    )
    sharding = NamedSharding(mesh, spec)
    ex = (sharded, in_names, out_names, out_avals, sharding)
    _program_cache[key] = ex
    return ex


def _run_device(args_dev, rt):
    import jax

    sharded, in_names, out_names, out_avals, sharding = _get_executor(rt)
    # the kernel fully overwrites all outputs; donate last call's buffers
    prev = _program_cache.get(("outs", rt))
    if prev is None:
        prev = tuple(jax.device_put(
            np.zeros((B * av.shape[0], *av.shape[1:]), av.dtype), sharding)
            for av in out_avals)
    outs_dev = sharded(*args_dev, *prev)
    _program_cache[("outs", rt)] = outs_dev
    return {n: np.array(a).reshape(B, *out_avals[i].shape)
            for i, (n, a) in enumerate(zip(out_names, outs_dev))}


def kernel(distance_matrix, max_dist, time_window_compatibility, depot,
           num_neighbors_encoder):
    import jax

    distance_matrix = np.asarray(distance_matrix, dtype=np.float32)
    time_window_compatibility = np.asarray(time_window_compatibility,
                                           dtype=np.int32)
    depot = np.asarray(depot, dtype=np.int32)
    max_dist = np.asarray(max_dist, dtype=np.float32).reshape(B)
    assert int(np.asarray(num_neighbors_encoder)) == K
    assert distance_matrix.shape == (B, N, N)

    nd_counts = [(depot[b] == 0).sum() for b in range(B)]
    rt = max(1, int(-(-max(nd_counts) // P)))  # row-tiles of 128, >= 1
    C = min(rt * P, N)

    not_eye = ~np.eye(N, dtype=bool)
    preps = [_prep_core(distance_matrix[b], time_window_compatibility[b],
                        depot[b], rt, not_eye) for b in range(B)]
    sharded, in_names, out_names, out_avals, sharding = _get_executor(rt)
    assert in_names == ["x"], in_names
    concat_x = np.concatenate([p[0] for p in preps], axis=0)
    args_dev = [jax.device_put(concat_x, sharding)]

    rng = np.random.default_rng(0)
    for attempt in range(3):
        by_name = _run_device(args_dev, rt)
        raw = by_name["mask"]     # [B, R, C] uint8: 1 sel, 0/255 unsel
        cnt_rest = by_name["cnt"]  # [B, P, rt]
        acc = by_name["acc"]       # [B, P, rt]: #sel - #unsel over stored C
        t16 = by_name["t16"]       # [B, P, rt]

        out = np.zeros((B, N, N), np.float32)
        ar = np.arange(N)
        for b in range(B):
            _, nd, colperm = preps[b]
            R = len(nd)
            full = np.zeros((R, N), np.float32)
            full[:, colperm[:C]] = (raw[b, :R, :] == 1)
            out[b, nd] = full
            dep_mask = depot[b] == 1
            out[b, dep_mask, :] = 1.0
            out[b, :, dep_mask] = 1.0
            out[b, ar, ar] = 1.0

            # exact repair of rows whose t16 is unreliable: count != 16
            # (bf16 tie at the 16/17 boundary, fold collision, or chunk
            # coverage miss all push the count off 16), < 16 eligible
            # neighbors (t16 = -3 sentinel), or |t16| below the eps guard
            count = (acc[b] + np.float32(C)) * np.float32(0.5) + cnt_rest[b]
            rr = np.arange(R)
            pp, tt = rr % P, rr // P
            bad = ((count[pp, tt] != np.float32(K))
                   | (t16[b][pp, tt] <= -2.0)
                   | (np.abs(t16[b][pp, tt]) < 1e-3))
            for r in np.flatnonzero(bad):
                i = int(nd[r])
                out[b, i] = _repair_row(
                    distance_matrix[b, i], time_window_compatibility[b, i],
                    depot[b], max_dist[b], i,
                )

        # audit: recompute a random sample of rows exactly on host; any
        # mismatch indicates a transient device glitch -> rerun the call
        ok = True
        for _ in range(192):
            b = int(rng.integers(B))
            i = int(rng.integers(N))
            exp = _repair_row(
                distance_matrix[b, i], time_window_compatibility[b, i],
                depot[b], max_dist[b], i,
            )
            if not np.array_equal(out[b, i], exp):
                ok = False
                break
        if ok:
            return out
    return out


# revision 8
# speedup vs baseline: 3.6397x; 1.0593x over previous
"""Trainium2 Bass kernel for nn_Actor_56916906607124 (compute_encoder_mask).

Computation (per batch instance b, row i):
  mask[b,i,j] = 1 iff  (j is among the 16 nearest time-window-compatible,
                        non-diagonal neighbors of i)  OR depot[b,i]  OR
                        depot[b,j]  OR i == j.

Sharding: pure data parallelism -- batch B=8 across 8 NeuronCores, one
instance per core.  No collectives.

Key structural facts exploited:
  * depot rows are all-ones and depot columns are all-ones in the output,
    independent of the KNN result.  Only non-depot rows (~1024 of 2048 per
    instance) need the device; the host memsets the rest while unsharding.
  * the selection key x = (twc && !diag) ? -d : -3 folds both inputs into a
    single bf16 tensor: eligible j have x = -d in (-1, 0], blocked j sit at
    -3, and the 16 nearest eligible neighbors are exactly the top-16 of x.
    bf16 rounding is monotone, so the bf16 top-16 equals the f32 top-16
    unless two values collide at the 16/17 boundary -- which the count
    check flags for exact host repair.

Per-core device program (R=1152 padded non-depot rows, 9 tiles of 128):
  DMA   : x tile [128,2048] bf16 in (4096 B/row descriptors, full rate).
  DVE   : folded = max(x[:, :1024], x[:, 1024:])  (bf16 2x mode; the Pool
          engine cannot run ALU ops on core v3);
          4x max8 over 256-wide chunks of folded -> 32 candidates;
          max8 -> top-8, match_replace, max8 -> ranks 9..16 => t16;
          bias = -t16 + eps;  is_ge count over the 896 non-stored cols
          (4x DVE mode: all-bf16 packed operands).
  ACT   : Sign(x + bias) SBUF->SBUF straight to uint8 over the 1152 stored
          cols (negatives wrap to 255; host maps ==1) with the accumulator
          shipping #sel - #unsel per row.
  DMA   : mask tile [128,1152] uint8 out on the scalar queue.

Host flags rows with count != 16 (boundary tie in bf16, fold collision, or
chunk-coverage miss -- any wrong t16 shifts the count off 16), t16 <= -2
(fewer than 16 eligible) or |t16| < 1e-3 (eps-guard margin), and recomputes
exactly those rows in f32 numpy.  ~950 of ~8100 rows on the seed-0 data;
verified to cover every differing row.
"""

from contextlib import ExitStack

import numpy as np

import concourse.bass as bass
import concourse.mybir as mybir
from concourse import bacc, tile

B, N, P = 8, 2048, 128
K = 16
EPS = 1e-7
f32 = mybir.dt.float32
bf16 = mybir.dt.bfloat16
u8 = mybir.dt.uint8
Alu = mybir.AluOpType
Act = mybir.ActivationFunctionType

_program_cache = {}


def build_program(rt=9):
    """Device program for RT row-tiles of 128 non-depot rows."""
    key = ("nc", rt)
    if key in _program_cache:
        return _program_cache[key]
    R = rt * P          # padded non-depot rows
    C = min(R, N)       # stored (non-depot-first) columns
    REST = N - C        # trailing depot columns: counted, not stored

    nc = bacc.Bacc()
    x_h = nc.declare_dram_parameter("x", [R, N], bf16, isOutput=False)
    mask_h = nc.declare_dram_parameter("mask", [R, C], u8, isOutput=True)
    # stats columns: [0:rt] = rest-col count, [rt:2rt] = stored acc
    # (#sel - #unsel), [2rt:3rt] = ACT bias (host recovers t16 ~ EPS - bias)
    stats_h = nc.declare_dram_parameter("stats", [P, 3 * rt], f32,
                                        isOutput=True)

    H = N // 2
    with ExitStack() as ctx:
        tc = ctx.enter_context(tile.TileContext(nc))
        const = ctx.enter_context(tc.tile_pool(name="const", bufs=1))
        inp = ctx.enter_context(tc.tile_pool(name="inp", bufs=5))
        fold = ctx.enter_context(tc.tile_pool(name="fold", bufs=3))
        outp = ctx.enter_context(tc.tile_pool(name="outp", bufs=rt))
        small = ctx.enter_context(tc.tile_pool(name="small", bufs=4))

        v8ball = const.tile([P, 8 * rt], f32)
        stats_s = const.tile([P, 3 * rt], f32)

        pending_stores = []
        for r in range(rt):
            rows = slice(r * P, (r + 1) * P)
            x_t = inp.tile([P, N], bf16, tag="x")
            f_t = fold.tile([P, H], bf16, tag="f")
            if r == 0:
                # ramp: tile 0 loads in column pieces spread across three
                # HWDGE queues so the configs overlap, and folds in halves
                # so the DVE starts after the first two pieces land
                nc.sync.dma_start(x_t[:, 0:512], x_h[rows, 0:512])
                nc.scalar.dma_start(x_t[:, H : H + 512], x_h[rows, H : H + 512])
                nc.sync.dma_start(x_t[:, 512:H], x_h[rows, 512:H])
                nc.scalar.dma_start(x_t[:, H + 512 :], x_h[rows, H + 512 :])
                nc.vector.tensor_tensor(
                    f_t[:, 0:512], x_t[:, 0:512], x_t[:, H : H + 512], Alu.max)
            else:
                nc.sync.dma_start(x_t[:], x_h[rows, :])
                # folded[j] = max(x[j], x[j+1024]): any top-16 member of x
                # survives into folded unless its partner also is one (fold
                # collision) -- then t16 comes out low and the count flags.
                nc.vector.tensor_tensor(
                    f_t[:], x_t[:, :H], x_t[:, H:], Alu.max)
            # per-chunk top-8 of folded -> 32 candidates
            cand = small.tile([P, 32], f32, tag="cand")
            for c in range(4):
                if r == 0 and c == 2:
                    nc.vector.tensor_tensor(
                        f_t[:, 512:], x_t[:, 512:H], x_t[:, H + 512 :],
                        Alu.max)
                nc.vector.max(cand[:, c * 8 : (c + 1) * 8],
                              f_t[:, c * 256 : (c + 1) * 256])
            v8a = small.tile([P, 8], f32, tag="v8a")
            nc.vector.max(v8a[:], cand[:])
            cand2 = small.tile([P, 32], f32, tag="cand2")
            nc.vector.match_replace(cand2[:], v8a[:], cand[:], -1e30)
            v8b = v8ball[:, r * 8 : (r + 1) * 8]
            nc.vector.max(v8b, cand2[:])
            t16 = v8ball[:, r * 8 + 7 : r * 8 + 8]
            # ACT bias: -t16 + EPS (EPS < any bf16 gap at |t16| >= 1e-3, so
            # Sign(x + bias) > 0  <=>  x >= t16; |t16| < 1e-3 rows flagged)
            bias = stats_s[:, 2 * rt + r : 2 * rt + r + 1]
            nc.vector.tensor_scalar(bias, t16, -1.0, EPS, Alu.mult, Alu.add)
            # stored mask: Sign gives 1 / 0 / -1(->255 as uint8); the
            # accumulator ships  #sel - #unsel  so count = (acc + C) / 2
            out_t = outp.tile([P, C], u8, tag="out")
            nc.scalar.activation(out_t[:], x_t[:, :C], Act.Sign, bias=bias,
                                 accum_out=stats_s[:, rt + r : rt + r + 1])
            if REST:
                # top-16 membership count over the non-stored (depot) cols;
                # in-place over x so no extra SBUF, 4x DVE mode (all bf16)
                nc.vector.tensor_scalar(
                    x_t[:, C:], x_t[:, C:], t16, None, Alu.is_ge, Alu.add,
                    accum_out=stats_s[:, r : r + 1])
            else:
                nc.vector.memset(stats_s[:, r : r + 1], 0.0)
            pending_stores.append((rows, out_t))

        # mask stores ride the sync queue AFTER every load: each store's
        # dependency wait blocks the issuing sequencer, so putting them on
        # the scalar queue would stall the next tile's Sign dispatch
        for rows, out_t in pending_stores:
            nc.sync.dma_start(mask_h[rows, :], out_t[:])
        nc.scalar.dma_start(stats_h[:, :], stats_s[:])

    nc.compile()
    _program_cache[key] = nc
    return nc


def _repair_row(d_row, twc_row, depot_b, max_dist_b, i):
    """Exact float32 re-computation of reference row i (handles ties)."""
    n = d_row.shape[0]
    m = (twc_row == 0).astype(np.float32)
    m[i] = np.float32(1.0)
    big = (m * np.float32(max_dist_b)) * np.float32(10.0)
    dist = d_row * (np.float32(1.0) - m) + big
    idx = np.argsort(dist, kind="stable")[:K]
    knn = np.zeros(n, np.float32)
    knn[idx] = 1.0
    knn *= (twc_row == 1)
    dep = (depot_b + depot_b[i]) > 0
    out = ((knn > 0) | dep | (np.arange(n) == i)).astype(np.float32)
    return out


def _prep_core(d_b, twc_b, depot_b, rt, not_eye):
    """Build the per-core compacted selection-key tensor + index maps."""
    R = rt * P
    bf = mybir.dt.np(bf16)
    nd = np.flatnonzero(depot_b == 0)
    dep = np.flatnonzero(depot_b == 1)
    colperm = np.concatenate([nd, dep])
    xf = np.where((twc_b == 1) & not_eye, -d_b, np.float32(-3.0))
    xc = np.full((R, N), np.float32(-3.0), np.float32)
    xc[: len(nd)] = xf[nd][:, colperm]
    return xc.astype(bf), nd, colperm


def _get_executor(rt=9):
    """Build the 8-core shard_map executable once (mirrors
    bass2jax.run_bass_via_pjrt, but cached so repeat calls skip retracing)."""
    key = ("exec", rt)
    if key in _program_cache:
        return _program_cache[key]
    import jax
    from jax.sharding import Mesh, NamedSharding, PartitionSpec
    from jax.experimental.shard_map import shard_map
    from concourse import bass2jax
    from concourse.bass2jax import _bass_exec_p, install_neuronx_cc_hook

    nc = build_program(rt)
    install_neuronx_cc_hook()
    partition_name = (nc.partition_id_tensor.name
                      if nc.partition_id_tensor else None)
    in_names, out_names, out_avals = [], [], []
    for alloc in nc.m.functions[0].allocations:
        if not isinstance(alloc, mybir.MemoryLocationSet):
            continue
        name = alloc.memorylocations[0].name
        if alloc.kind == "ExternalInput":
            if name != partition_name:
                in_names.append(name)
        elif alloc.kind == "ExternalOutput":
            out_names.append(name)
            out_avals.append(jax.core.ShapedArray(
                tuple(alloc.tensor_shape), mybir.dt.np(alloc.dtype)))
    all_in_names = list(in_names) + list(out_names)
    if partition_name is not None:
        all_in_names.append(partition_name)

    def _body(*args):
        operands = list(args)
        if partition_name is not None:
            operands.append(bass2jax.partition_id_tensor())
        return tuple(_bass_exec_p.bind(
            *operands,
            out_avals=tuple(out_avals),
            in_names=tuple(all_in_names),
            out_names=tuple(out_names),
            lowering_input_output_aliases=(),
            sim_require_finite=True,
            sim_require_nnan=True,
            nc=nc,
        ))

    devices = jax.devices()[:B]
    mesh = Mesh(np.asarray(devices), ("core",))
    spec = PartitionSpec("core")
    n_io = len(in_names) + len(out_names)
    sharded = jax.jit(
        shard_map(_body, mesh=mesh, in_specs=(spec,) * n_io,
                  out_specs=(spec,) * len(out_names), check_rep=False),
        donate_argnums=tuple(range(len(in_names), n_io)), keep_unused=True,
    )
    sharding = NamedSharding(mesh, spec)
    ex = (sharded, in_names, out_names, out_avals, sharding)
    _program_cache[key] = ex
    return ex


def _run_device(args_dev, rt):
    import jax

    sharded, in_names, out_names, out_avals, sharding = _get_executor(rt)
    # the kernel fully overwrites all outputs; donate last call's buffers
    prev = _program_cache.get(("outs", rt))
    if prev is None:
        prev = tuple(jax.device_put(
            np.zeros((B * av.shape[0], *av.shape[1:]), av.dtype), sharding)
            for av in out_avals)
    outs_dev = sharded(*args_dev, *prev)
    _program_cache[("outs", rt)] = outs_dev
    return {n: np.array(a).reshape(B, *out_avals[i].shape)
            for i, (n, a) in enumerate(zip(out_names, outs_dev))}


def kernel(distance_matrix, max_dist, time_window_compatibility, depot,
           num_neighbors_encoder):
    import jax

    distance_matrix = np.asarray(distance_matrix, dtype=np.float32)
    time_window_compatibility = np.asarray(time_window_compatibility,
                                           dtype=np.int32)
    depot = np.asarray(depot, dtype=np.int32)
    max_dist = np.asarray(max_dist, dtype=np.float32).reshape(B)
    assert int(np.asarray(num_neighbors_encoder)) == K
    assert distance_matrix.shape == (B, N, N)

    nd_counts = [(depot[b] == 0).sum() for b in range(B)]
    rt = max(1, int(-(-max(nd_counts) // P)))  # row-tiles of 128, >= 1
    C = min(rt * P, N)

    not_eye = ~np.eye(N, dtype=bool)
    preps = [_prep_core(distance_matrix[b], time_window_compatibility[b],
                        depot[b], rt, not_eye) for b in range(B)]
    sharded, in_names, out_names, out_avals, sharding = _get_executor(rt)
    assert in_names == ["x"], in_names
    concat_x = np.concatenate([p[0] for p in preps], axis=0)
    args_dev = [jax.device_put(concat_x, sharding)]

    rng = np.random.default_rng(0)
    for attempt in range(3):
        by_name = _run_device(args_dev, rt)
        raw = by_name["mask"]      # [B, R, C] uint8: 1 sel, 0/255 unsel
        stats = by_name["stats"]   # [B, P, 3*rt]
        cnt_rest = stats[:, :, :rt]
        acc = stats[:, :, rt : 2 * rt]      # #sel - #unsel over stored C
        t16 = np.float32(EPS) - stats[:, :, 2 * rt :]  # ~t16 (+-1 ulp)

        out = np.zeros((B, N, N), np.float32)
        ar = np.arange(N)
        for b in range(B):
            _, nd, colperm = preps[b]
            R = len(nd)
            full = np.zeros((R, N), np.float32)
            full[:, colperm[:C]] = (raw[b, :R, :] == 1)
            out[b, nd] = full
            dep_mask = depot[b] == 1
            out[b, dep_mask, :] = 1.0
            out[b, :, dep_mask] = 1.0
            out[b, ar, ar] = 1.0

            # exact repair of rows whose t16 is unreliable: count != 16
            # (bf16 tie at the 16/17 boundary, fold collision, or chunk
            # coverage miss all push the count off 16), < 16 eligible
            # neighbors (t16 = -3 sentinel), or |t16| below the eps guard
            count = (acc[b] + np.float32(C)) * np.float32(0.5) + cnt_rest[b]
            rr = np.arange(R)
            pp, tt = rr % P, rr // P
            bad = ((count[pp, tt] != np.float32(K))
                   | (t16[b][pp, tt] <= -1.5)
                   | (np.abs(t16[b][pp, tt]) < 1e-3))
            for r in np.flatnonzero(bad):
                i = int(nd[r])
                out[b, i] = _repair_row(
                    distance_matrix[b, i], time_window_compatibility[b, i],
                    depot[b], max_dist[b], i,
                )

        # audit: recompute a random sample of rows exactly on host; any
        # mismatch indicates a transient device glitch -> rerun the call
        ok = True
        for _ in range(192):
            b = int(rng.integers(B))
            i = int(rng.integers(N))
            exp = _repair_row(
                distance_matrix[b, i], time_window_compatibility[b, i],
                depot[b], max_dist[b], i,
            )
            if not np.array_equal(out[b, i], exp):
                ok = False
                break
        if ok:
            return out
    return out


# revision 11
# speedup vs baseline: 3.7596x; 1.0329x over previous
"""Trainium2 Bass kernel for nn_Actor_56916906607124 (compute_encoder_mask).

Computation (per batch instance b, row i):
  mask[b,i,j] = 1 iff  (j is among the 16 nearest time-window-compatible,
                        non-diagonal neighbors of i)  OR depot[b,i]  OR
                        depot[b,j]  OR i == j.

Sharding: pure data parallelism -- batch B=8 across 8 NeuronCores, one
instance per core.  No collectives.

Key structural facts exploited:
  * depot rows are all-ones and depot columns are all-ones in the output,
    independent of the KNN result.  Only non-depot rows (~1024 of 2048 per
    instance) need the device; the host memsets the rest while unsharding.
  * the selection key x = (twc && !diag) ? -d : -3 folds both inputs into a
    single bf16 tensor: eligible j have x = -d in (-1, 0], blocked j sit at
    -3, and the 16 nearest eligible neighbors are exactly the top-16 of x.
    bf16 rounding is monotone, so the bf16 top-16 equals the f32 top-16
    unless two values collide at the 16/17 boundary -- which the count
    check flags for exact host repair.

Per-core device program (R=1152 padded non-depot rows, 9 tiles of 128):
  DMA   : x tile [128,2048] bf16 in (4096 B/row descriptors, full rate).
  DVE   : folded = max(x[:, :1024], x[:, 1024:])  (bf16 2x mode; the Pool
          engine cannot run ALU ops on core v3);
          4x max8 over 256-wide chunks of folded -> 32 candidates;
          max8 -> top-8, match_replace, max8 -> ranks 9..16 => t16;
          bias = -t16 + eps;  is_ge count over the 896 non-stored cols
          (4x DVE mode: all-bf16 packed operands).
  ACT   : Sign(x + bias) SBUF->SBUF straight to uint8 over the 1152 stored
          cols (negatives wrap to 255; host maps ==1) with the accumulator
          shipping #sel - #unsel per row.
  DMA   : mask tile [128,1152] uint8 out on the scalar queue.

Host flags rows with count != 16 (boundary tie in bf16, fold collision, or
chunk-coverage miss -- any wrong t16 shifts the count off 16), t16 <= -2
(fewer than 16 eligible) or |t16| < 1e-3 (eps-guard margin), and recomputes
exactly those rows in f32 numpy.  ~950 of ~8100 rows on the seed-0 data;
verified to cover every differing row.
"""

from contextlib import ExitStack

import numpy as np

import concourse.bass as bass
import concourse.mybir as mybir
from concourse import bacc, tile

B, N, P = 8, 2048, 128
K = 16
EPS = 1e-7
f32 = mybir.dt.float32
bf16 = mybir.dt.bfloat16
u8 = mybir.dt.uint8
Alu = mybir.AluOpType
Act = mybir.ActivationFunctionType

_program_cache = {}


def build_program(rt=9):
    """Device program for RT row-tiles of 128 non-depot rows."""
    key = ("nc", rt)
    if key in _program_cache:
        return _program_cache[key]
    R = rt * P          # padded non-depot rows
    C = min(R, N)       # stored (non-depot-first) columns
    REST = N - C        # trailing depot columns: counted, not stored

    nc = bacc.Bacc()
    x_h = nc.declare_dram_parameter("x", [R, N], bf16, isOutput=False)
    mask_h = nc.declare_dram_parameter("mask", [R, C], u8, isOutput=True)
    # stats columns: [0:rt] = rest-col count, [rt:2rt] = stored acc
    # (#sel - #unsel), [2rt:3rt] = ACT bias (host recovers t16 ~ EPS - bias),
    # [3rt] = second-half acc of the split last tile
    stats_h = nc.declare_dram_parameter("stats", [P, 3 * rt + 1], f32,
                                        isOutput=True)

    H = N // 2
    with ExitStack() as ctx:
        tc = ctx.enter_context(tile.TileContext(nc))
        const = ctx.enter_context(tc.tile_pool(name="const", bufs=1))
        inp = ctx.enter_context(tc.tile_pool(name="inp", bufs=5))
        fold = ctx.enter_context(tc.tile_pool(name="fold", bufs=3))
        outp = ctx.enter_context(tc.tile_pool(name="outp", bufs=rt))
        small = ctx.enter_context(tc.tile_pool(name="small", bufs=4))

        v8ball = const.tile([P, 8 * rt], f32)
        stats_s = const.tile([P, 3 * rt + 1], f32)

        pending_stores = []
        for r in range(rt):
            rows = slice(r * P, (r + 1) * P)
            x_t = inp.tile([P, N], bf16, tag="x")
            f_t = fold.tile([P, H], bf16, tag="f")
            if r == 0:
                # ramp: tile 0 loads in column pieces spread across both
                # HWDGE queues so the configs overlap, and fold1 runs in
                # halves so the DVE starts after the first two pieces land
                nc.sync.dma_start(x_t[:, 0:512], x_h[rows, 0:512])
                nc.scalar.dma_start(x_t[:, H : H + 512], x_h[rows, H : H + 512])
                nc.sync.dma_start(x_t[:, 512:H], x_h[rows, 512:H])
                nc.scalar.dma_start(x_t[:, H + 512 :], x_h[rows, H + 512 :])
                nc.vector.tensor_tensor(
                    f_t[:, 0:512], x_t[:, 0:512], x_t[:, H : H + 512], Alu.max)
                nc.vector.tensor_tensor(
                    f_t[:, 512:], x_t[:, 512:H], x_t[:, H + 512 :], Alu.max)
            else:
                nc.sync.dma_start(x_t[:], x_h[rows, :])
                # fold1[j] = max(x[j], x[j+1024]): any top-16 member of x
                # survives folding unless its partner also is one (fold
                # collision) -- then t16 comes out low and the count flags.
                nc.vector.tensor_tensor(
                    f_t[:], x_t[:, :H], x_t[:, H:], Alu.max)
            # fold2 (in place): slot j covers {j, j+512, j+1024, j+1536}
            nc.vector.tensor_tensor(
                f_t[:, 0:512], f_t[:, 0:512], f_t[:, 512:], Alu.max)
            # per-chunk top-8 of the 512 fold2 slots -> 32 candidates
            cand = small.tile([P, 32], f32, tag="cand")
            for c in range(4):
                nc.vector.max(cand[:, c * 8 : (c + 1) * 8],
                              f_t[:, c * 128 : (c + 1) * 128])
            v8a = small.tile([P, 8], f32, tag="v8a")
            nc.vector.max(v8a[:], cand[:])
            cand2 = small.tile([P, 32], f32, tag="cand2")
            nc.vector.match_replace(cand2[:], v8a[:], cand[:], -1e30)
            v8b = v8ball[:, r * 8 : (r + 1) * 8]
            nc.vector.max(v8b, cand2[:])
            t16 = v8ball[:, r * 8 + 7 : r * 8 + 8]
            # ACT bias: -t16 + EPS (EPS < any bf16 gap at |t16| >= 1e-3, so
            # Sign(x + bias) > 0  <=>  x >= t16; |t16| < 1e-3 rows flagged)
            bias = stats_s[:, 2 * rt + r : 2 * rt + r + 1]
            nc.vector.tensor_scalar(bias, t16, -1.0, EPS, Alu.mult, Alu.add)
            # stored mask: Sign gives 1 / 0 / -1(->255 as uint8); the
            # accumulator ships  #sel - #unsel  so count = (acc + C) / 2.
            # Last tile runs in halves so its store drains while the second
            # half is still on the ACT engine.
            out_t = outp.tile([P, C], u8, tag="out")
            if r == rt - 1:
                hc = (C // 2 + 127) & ~127
                nc.scalar.activation(out_t[:, :hc], x_t[:, :hc], Act.Sign,
                                     bias=bias,
                                     accum_out=stats_s[:, rt + r : rt + r + 1])
                nc.scalar.activation(out_t[:, hc:], x_t[:, hc:C], Act.Sign,
                                     bias=bias,
                                     accum_out=stats_s[:, 3 * rt : 3 * rt + 1])
                pending_stores.append((rows, out_t, hc))
            else:
                nc.scalar.activation(out_t[:], x_t[:, :C], Act.Sign, bias=bias,
                                     accum_out=stats_s[:, rt + r : rt + r + 1])
                pending_stores.append((rows, out_t, None))
            if REST:
                # top-16 membership count over the non-stored (depot) cols;
                # in-place over x so no extra SBUF, 4x DVE mode (all bf16)
                nc.vector.tensor_scalar(
                    x_t[:, C:], x_t[:, C:], t16, None, Alu.is_ge, Alu.add,
                    accum_out=stats_s[:, r : r + 1])
            else:
                nc.vector.memset(stats_s[:, r : r + 1], 0.0)

        # mask stores ride the sync queue AFTER every load: each store's
        # dependency wait blocks the issuing sequencer, so putting them on
        # the scalar queue would stall the next tile's Sign dispatch
        for rows, out_t, hc in pending_stores:
            if hc is None:
                nc.sync.dma_start(mask_h[rows, :], out_t[:])
            else:
                nc.sync.dma_start(mask_h[rows, :hc], out_t[:, :hc])
                nc.sync.dma_start(mask_h[rows, hc:], out_t[:, hc:])
        nc.scalar.dma_start(stats_h[:, :], stats_s[:])

    nc.compile()
    _program_cache[key] = nc
    return nc


def _repair_row(d_row, twc_row, depot_b, max_dist_b, i):
    """Exact float32 re-computation of reference row i (handles ties)."""
    n = d_row.shape[0]
    m = (twc_row == 0).astype(np.float32)
    m[i] = np.float32(1.0)
    big = (m * np.float32(max_dist_b)) * np.float32(10.0)
    dist = d_row * (np.float32(1.0) - m) + big
    idx = np.argsort(dist, kind="stable")[:K]
    knn = np.zeros(n, np.float32)
    knn[idx] = 1.0
    knn *= (twc_row == 1)
    dep = (depot_b + depot_b[i]) > 0
    out = ((knn > 0) | dep | (np.arange(n) == i)).astype(np.float32)
    return out


def _prep_core(d_b, twc_b, depot_b, rt, not_eye):
    """Build the per-core compacted selection-key tensor + index maps."""
    R = rt * P
    bf = mybir.dt.np(bf16)
    nd = np.flatnonzero(depot_b == 0)
    dep = np.flatnonzero(depot_b == 1)
    colperm = np.concatenate([nd, dep])
    xf = np.where((twc_b == 1) & not_eye, -d_b, np.float32(-3.0))
    xc = np.full((R, N), np.float32(-3.0), np.float32)
    xc[: len(nd)] = xf[nd][:, colperm]
    return xc.astype(bf), nd, colperm


def _get_executor(rt=9):
    """Build the 8-core shard_map executable once (mirrors
    bass2jax.run_bass_via_pjrt, but cached so repeat calls skip retracing)."""
    key = ("exec", rt)
    if key in _program_cache:
        return _program_cache[key]
    import jax
    from jax.sharding import Mesh, NamedSharding, PartitionSpec
    from jax.experimental.shard_map import shard_map
    from concourse import bass2jax
    from concourse.bass2jax import _bass_exec_p, install_neuronx_cc_hook

    nc = build_program(rt)
    install_neuronx_cc_hook()
    partition_name = (nc.partition_id_tensor.name
                      if nc.partition_id_tensor else None)
    in_names, out_names, out_avals = [], [], []
    for alloc in nc.m.functions[0].allocations:
        if not isinstance(alloc, mybir.MemoryLocationSet):
            continue
        name = alloc.memorylocations[0].name
        if alloc.kind == "ExternalInput":
            if name != partition_name:
                in_names.append(name)
        elif alloc.kind == "ExternalOutput":
            out_names.append(name)
            out_avals.append(jax.core.ShapedArray(
                tuple(alloc.tensor_shape), mybir.dt.np(alloc.dtype)))
    all_in_names = list(in_names) + list(out_names)
    if partition_name is not None:
        all_in_names.append(partition_name)

    def _body(*args):
        operands = list(args)
        if partition_name is not None:
            operands.append(bass2jax.partition_id_tensor())
        return tuple(_bass_exec_p.bind(
            *operands,
            out_avals=tuple(out_avals),
            in_names=tuple(all_in_names),
            out_names=tuple(out_names),
            lowering_input_output_aliases=(),
            sim_require_finite=True,
            sim_require_nnan=True,
            nc=nc,
        ))

    devices = jax.devices()[:B]
    mesh = Mesh(np.asarray(devices), ("core",))
    spec = PartitionSpec("core")
    n_io = len(in_names) + len(out_names)
    sharded = jax.jit(
        shard_map(_body, mesh=mesh, in_specs=(spec,) * n_io,
                  out_specs=(spec,) * len(out_names), check_rep=False),
        donate_argnums=tuple(range(len(in_names), n_io)), keep_unused=True,
    )
    sharding = NamedSharding(mesh, spec)
    ex = (sharded, in_names, out_names, out_avals, sharding)
    _program_cache[key] = ex
    return ex


def _run_device(args_dev, rt):
    import jax

    sharded, in_names, out_names, out_avals, sharding = _get_executor(rt)
    # the kernel fully overwrites all outputs; donate last call's buffers
    prev = _program_cache.get(("outs", rt))
    if prev is None:
        prev = tuple(jax.device_put(
            np.zeros((B * av.shape[0], *av.shape[1:]), av.dtype), sharding)
            for av in out_avals)
    outs_dev = sharded(*args_dev, *prev)
    _program_cache[("outs", rt)] = outs_dev
    return {n: np.array(a).reshape(B, *out_avals[i].shape)
            for i, (n, a) in enumerate(zip(out_names, outs_dev))}


def kernel(distance_matrix, max_dist, time_window_compatibility, depot,
           num_neighbors_encoder):
    import jax

    distance_matrix = np.asarray(distance_matrix, dtype=np.float32)
    time_window_compatibility = np.asarray(time_window_compatibility,
                                           dtype=np.int32)
    depot = np.asarray(depot, dtype=np.int32)
    max_dist = np.asarray(max_dist, dtype=np.float32).reshape(B)
    assert int(np.asarray(num_neighbors_encoder)) == K
    assert distance_matrix.shape == (B, N, N)

    nd_counts = [(depot[b] == 0).sum() for b in range(B)]
    rt = max(1, int(-(-max(nd_counts) // P)))  # row-tiles of 128, >= 1
    C = min(rt * P, N)

    not_eye = ~np.eye(N, dtype=bool)
    preps = [_prep_core(distance_matrix[b], time_window_compatibility[b],
                        depot[b], rt, not_eye) for b in range(B)]
    sharded, in_names, out_names, out_avals, sharding = _get_executor(rt)
    assert in_names == ["x"], in_names
    concat_x = np.concatenate([p[0] for p in preps], axis=0)
    args_dev = [jax.device_put(concat_x, sharding)]

    rng = np.random.default_rng(0)
    for attempt in range(3):
        by_name = _run_device(args_dev, rt)
        raw = by_name["mask"]      # [B, R, C] uint8: 1 sel, 0/255 unsel
        stats = by_name["stats"]   # [B, P, 3*rt+1]
        cnt_rest = stats[:, :, :rt]
        acc = stats[:, :, rt : 2 * rt].copy()  # #sel - #unsel over stored C
        acc[:, :, rt - 1] += stats[:, :, 3 * rt]  # split last tile
        t16 = np.float32(EPS) - stats[:, :, 2 * rt : 3 * rt]  # ~t16 (+-1 ulp)

        out = np.zeros((B, N, N), np.float32)
        ar = np.arange(N)
        for b in range(B):
            _, nd, colperm = preps[b]
            R = len(nd)
            full = np.zeros((R, N), np.float32)
            full[:, colperm[:C]] = (raw[b, :R, :] == 1)
            out[b, nd] = full
            dep_mask = depot[b] == 1
            out[b, dep_mask, :] = 1.0
            out[b, :, dep_mask] = 1.0
            out[b, ar, ar] = 1.0

            # exact repair of rows whose t16 is unreliable: count != 16
            # (bf16 tie at the 16/17 boundary, fold collision, or chunk
            # coverage miss all push the count off 16), < 16 eligible
            # neighbors (t16 = -3 sentinel), or |t16| below the eps guard
            count = (acc[b] + np.float32(C)) * np.float32(0.5) + cnt_rest[b]
            rr = np.arange(R)
            pp, tt = rr % P, rr // P
            bad = ((count[pp, tt] != np.float32(K))
                   | (t16[b][pp, tt] <= -1.5)
                   | (np.abs(t16[b][pp, tt]) < 1e-3))
            for r in np.flatnonzero(bad):
                i = int(nd[r])
                out[b, i] = _repair_row(
                    distance_matrix[b, i], time_window_compatibility[b, i],
                    depot[b], max_dist[b], i,
                )

        # audit: recompute a random sample of rows exactly on host; any
        # mismatch indicates a transient device glitch -> rerun the call
        ok = True
        for _ in range(192):
            b = int(rng.integers(B))
            i = int(rng.integers(N))
            exp = _repair_row(
                distance_matrix[b, i], time_window_compatibility[b, i],
                depot[b], max_dist[b], i,
            )
            if not np.array_equal(out[b, i], exp):
                ok = False
                break
        if ok:
            return out
    return out


# revision 18
# speedup vs baseline: 4.0748x; 1.0838x over previous
"""Trainium2 Bass kernel for nn_Actor_56916906607124 (compute_encoder_mask).

Computation (per batch instance b, row i):
  mask[b,i,j] = 1 iff  (j is among the 16 nearest time-window-compatible,
                        non-diagonal neighbors of i)  OR depot[b,i]  OR
                        depot[b,j]  OR i == j.

Sharding: pure data parallelism -- batch B=8 across 8 NeuronCores, one
instance per core.  No collectives.

Key structural facts exploited:
  * depot rows are all-ones and depot columns are all-ones in the output,
    independent of the KNN result.  Only non-depot rows (~1024 of 2048 per
    instance) need the device; the host memsets the rest while unsharding.
  * the selection key x = (twc && !diag) ? -d : -3 folds both inputs into a
    single bf16 tensor: eligible j have x = -d in (-1, 0], blocked j sit at
    -3, and the 16 nearest eligible neighbors are exactly the top-16 of x.
    bf16 rounding is monotone, so the bf16 top-16 equals the f32 top-16
    unless two values collide at the 16/17 boundary -- which the count
    check flags for exact host repair.

Per-core device program (R=1152 padded non-depot rows, 9 tiles of 128):
  DMA   : x tile [128,2048] bf16 in (4096 B/row descriptors, full rate).
  DVE   : folded = max(x[:, :1024], x[:, 1024:])  (bf16 2x mode; the Pool
          engine cannot run ALU ops on core v3);
          4x max8 over 256-wide chunks of folded -> 32 candidates;
          max8 -> top-8, match_replace, max8 -> ranks 9..16 => t16;
          bias = -t16 + eps;  is_ge count over the 896 non-stored cols
          (4x DVE mode: all-bf16 packed operands).
  ACT   : Sign(x + bias) SBUF->SBUF straight to uint8 over the 1152 stored
          cols (negatives wrap to 255; host maps ==1) with the accumulator
          shipping #sel - #unsel per row.
  DMA   : mask tile [128,1152] uint8 out on the scalar queue.

Host flags rows with count != 16 (boundary tie in bf16, fold collision, or
chunk-coverage miss -- any wrong t16 shifts the count off 16), t16 <= -2
(fewer than 16 eligible) or |t16| < 1e-3 (eps-guard margin), and recomputes
exactly those rows in f32 numpy.  ~950 of ~8100 rows on the seed-0 data;
verified to cover every differing row.
"""

from contextlib import ExitStack

import numpy as np

import concourse.bass as bass
import concourse.mybir as mybir
from concourse import bacc, tile

B, N, P = 8, 2048, 128
K = 16
EPS = 1e-7
f32 = mybir.dt.float32
bf16 = mybir.dt.bfloat16
u8 = mybir.dt.uint8
Alu = mybir.AluOpType
Act = mybir.ActivationFunctionType

_program_cache = {}


def build_program(rt=8, ct=9):
    """Device program for RT row-tiles of 128 non-depot rows; CT*128 stored
    (non-depot-first) columns."""
    key = ("nc", rt, ct)
    if key in _program_cache:
        return _program_cache[key]
    R = rt * P          # processed non-depot rows (leftover rows -> host)
    C = min(ct * P, N)  # stored (non-depot-first) columns
    REST = N - C        # trailing depot columns: counted, not stored
    K_ACT = max(0, rt - 3)  # tiles whose rest-count runs on ACT, not DVE

    nc = bacc.Bacc()
    x_h = nc.declare_dram_parameter("x", [R, N], bf16, isOutput=False)
    mask_h = nc.declare_dram_parameter("mask", [R, C], u8, isOutput=True)
    # stats columns: [0:rt] = rest-col count, [rt:2rt] = stored acc
    # (#sel - #unsel), [2rt:3rt] = ACT bias (host recovers t16 ~ EPS - bias),
    # [3rt] = second-half acc of the split last tile
    stats_h = nc.declare_dram_parameter("stats", [P, 3 * rt + 1], f32,
                                        isOutput=True)

    H = N // 2
    with ExitStack() as ctx:
        tc = ctx.enter_context(tile.TileContext(nc))
        const = ctx.enter_context(tc.tile_pool(name="const", bufs=1))
        inp = ctx.enter_context(tc.tile_pool(name="inp", bufs=5))
        fold = ctx.enter_context(tc.tile_pool(name="fold", bufs=3))
        outp = ctx.enter_context(tc.tile_pool(name="outp", bufs=rt))
        small = ctx.enter_context(tc.tile_pool(name="small", bufs=4))
        junk = ctx.enter_context(tc.tile_pool(name="junk", bufs=2))

        v8ball = const.tile([P, 8 * rt], f32)
        stats_s = const.tile([P, 3 * rt + 1], f32)

        pending_stores = []
        for r in range(rt):
            rows = slice(r * P, (r + 1) * P)
            x_t = inp.tile([P, N], bf16, tag="x")
            f_t = fold.tile([P, H], bf16, tag="f")
            if r == 0:
                # ramp: tile 0 loads in column pieces spread across both
                # HWDGE queues so the configs overlap, and fold1 runs in
                # halves so the DVE starts after the first two pieces land
                nc.sync.dma_start(x_t[:, 0:512], x_h[rows, 0:512])
                nc.scalar.dma_start(x_t[:, H : H + 512], x_h[rows, H : H + 512])
                nc.sync.dma_start(x_t[:, 512:H], x_h[rows, 512:H])
                nc.scalar.dma_start(x_t[:, H + 512 :], x_h[rows, H + 512 :])
                nc.vector.tensor_tensor(
                    f_t[:, 0:512], x_t[:, 0:512], x_t[:, H : H + 512], Alu.max)
                nc.vector.tensor_tensor(
                    f_t[:, 512:], x_t[:, 512:H], x_t[:, H + 512 :], Alu.max)
            else:
                nc.sync.dma_start(x_t[:], x_h[rows, :])
                # fold1[j] = max(x[j], x[j+1024]): any top-16 member of x
                # survives folding unless its partner also is one (fold
                # collision) -- then t16 comes out low and the count flags.
                nc.vector.tensor_tensor(
                    f_t[:], x_t[:, :H], x_t[:, H:], Alu.max)
            # fold2 (in place): slot j covers {j, j+512, j+1024, j+1536}
            nc.vector.tensor_tensor(
                f_t[:, 0:512], f_t[:, 0:512], f_t[:, 512:], Alu.max)
            # per-chunk top-8 of the 512 fold2 slots -> 32 candidates
            cand = small.tile([P, 32], f32, tag="cand")
            for c in range(4):
                nc.vector.max(cand[:, c * 8 : (c + 1) * 8],
                              f_t[:, c * 128 : (c + 1) * 128])
            v8a = small.tile([P, 8], f32, tag="v8a")
            nc.vector.max(v8a[:], cand[:])
            cand2 = small.tile([P, 32], f32, tag="cand2")
            nc.vector.match_replace(cand2[:], v8a[:], cand[:], -1e30)
            v8b = v8ball[:, r * 8 : (r + 1) * 8]
            nc.vector.max(v8b, cand2[:])
            t16 = v8ball[:, r * 8 + 7 : r * 8 + 8]
            # ACT bias: -t16 + EPS (EPS < any bf16 gap at |t16| >= 1e-3, so
            # Sign(x + bias) > 0  <=>  x >= t16; |t16| < 1e-3 rows flagged)
            bias = stats_s[:, 2 * rt + r : 2 * rt + r + 1]
            nc.vector.tensor_scalar(bias, t16, -1.0, EPS, Alu.mult, Alu.add)
            # stored mask: Sign gives 1 / 0 / -1(->255 as uint8); the
            # accumulator ships  #sel - #unsel  so count = (acc + C) / 2.
            # Last tile runs in halves so its store drains while the second
            # half is still on the ACT engine.
            out_t = outp.tile([P, C], u8, tag="out")
            if r == rt - 1:
                hc = (C // 2 + 127) & ~127
                nc.scalar.activation(out_t[:, :hc], x_t[:, :hc], Act.Sign,
                                     bias=bias,
                                     accum_out=stats_s[:, rt + r : rt + r + 1])
                nc.scalar.activation(out_t[:, hc:], x_t[:, hc:C], Act.Sign,
                                     bias=bias,
                                     accum_out=stats_s[:, 3 * rt : 3 * rt + 1])
                pending_stores.append((rows, out_t, hc))
            else:
                nc.scalar.activation(out_t[:], x_t[:, :C], Act.Sign, bias=bias,
                                     accum_out=stats_s[:, rt + r : rt + r + 1])
                pending_stores.append((rows, out_t, None))
            if not REST:
                nc.vector.memset(stats_s[:, r : r + 1], 0.0)
            elif r < K_ACT:
                # rest-col count on the otherwise-idle ACT: Sign into a junk
                # tile, accumulator ships #sel - #unsel (host adds REST, /2)
                j_t = junk.tile([P, REST], u8, tag="junk")
                nc.scalar.activation(j_t[:], x_t[:, C:], Act.Sign, bias=bias,
                                     accum_out=stats_s[:, r : r + 1])
            else:
                # last tiles keep the count on DVE so the ACT tail is short;
                # in-place over x so no extra SBUF, 4x DVE mode (all bf16)
                nc.vector.tensor_scalar(
                    x_t[:, C:], x_t[:, C:], t16, None, Alu.is_ge, Alu.add,
                    accum_out=stats_s[:, r : r + 1])

        # mask stores ride the sync queue AFTER every load: each store's
        # dependency wait blocks the issuing sequencer, so putting them on
        # the scalar queue would stall the next tile's Sign dispatch
        for rows, out_t, hc in pending_stores:
            if hc is None:
                nc.sync.dma_start(mask_h[rows, :], out_t[:])
            else:
                nc.sync.dma_start(mask_h[rows, :hc], out_t[:, :hc])
                nc.sync.dma_start(mask_h[rows, hc:], out_t[:, hc:])
        nc.scalar.dma_start(stats_h[:, :], stats_s[:])

    nc.compile()
    _program_cache[key] = nc
    return nc


def _repair_row(d_row, twc_row, depot_b, max_dist_b, i):
    """Exact float32 re-computation of reference row i (handles ties)."""
    n = d_row.shape[0]
    m = (twc_row == 0).astype(np.float32)
    m[i] = np.float32(1.0)
    big = (m * np.float32(max_dist_b)) * np.float32(10.0)
    dist = d_row * (np.float32(1.0) - m) + big
    idx = np.argsort(dist, kind="stable")[:K]
    knn = np.zeros(n, np.float32)
    knn[idx] = 1.0
    knn *= (twc_row == 1)
    dep = (depot_b + depot_b[i]) > 0
    out = ((knn > 0) | dep | (np.arange(n) == i)).astype(np.float32)
    return out


def _prep_core(d_b, twc_b, depot_b, rt, not_eye):
    """Build the per-core compacted selection-key tensor + index maps."""
    R = rt * P
    bf = mybir.dt.np(bf16)
    nd = np.flatnonzero(depot_b == 0)
    dep = np.flatnonzero(depot_b == 1)
    colperm = np.concatenate([nd, dep])
    xf = np.where((twc_b == 1) & not_eye, -d_b, np.float32(-3.0))
    xc = np.full((R, N), np.float32(-3.0), np.float32)
    nv = min(len(nd), R)
    xc[:nv] = xf[nd[:nv]][:, colperm]
    return xc.astype(bf), nd, colperm


def _get_executor(rt=8, ct=9):
    """Build the 8-core shard_map executable once (mirrors
    bass2jax.run_bass_via_pjrt, but cached so repeat calls skip retracing)."""
    key = ("exec", rt, ct)
    if key in _program_cache:
        return _program_cache[key]
    import jax
    from jax.sharding import Mesh, NamedSharding, PartitionSpec
    from jax.experimental.shard_map import shard_map
    from concourse import bass2jax
    from concourse.bass2jax import _bass_exec_p, install_neuronx_cc_hook

    nc = build_program(rt, ct)
    install_neuronx_cc_hook()
    partition_name = (nc.partition_id_tensor.name
                      if nc.partition_id_tensor else None)
    in_names, out_names, out_avals = [], [], []
    for alloc in nc.m.functions[0].allocations:
        if not isinstance(alloc, mybir.MemoryLocationSet):
            continue
        name = alloc.memorylocations[0].name
        if alloc.kind == "ExternalInput":
            if name != partition_name:
                in_names.append(name)
        elif alloc.kind == "ExternalOutput":
            out_names.append(name)
            out_avals.append(jax.core.ShapedArray(
                tuple(alloc.tensor_shape), mybir.dt.np(alloc.dtype)))
    all_in_names = list(in_names) + list(out_names)
    if partition_name is not None:
        all_in_names.append(partition_name)

    def _body(*args):
        operands = list(args)
        if partition_name is not None:
            operands.append(bass2jax.partition_id_tensor())
        return tuple(_bass_exec_p.bind(
            *operands,
            out_avals=tuple(out_avals),
            in_names=tuple(all_in_names),
            out_names=tuple(out_names),
            lowering_input_output_aliases=(),
            sim_require_finite=True,
            sim_require_nnan=True,
            nc=nc,
        ))

    devices = jax.devices()[:B]
    mesh = Mesh(np.asarray(devices), ("core",))
    spec = PartitionSpec("core")
    n_io = len(in_names) + len(out_names)
    sharded = jax.jit(
        shard_map(_body, mesh=mesh, in_specs=(spec,) * n_io,
                  out_specs=(spec,) * len(out_names), check_rep=False),
        donate_argnums=tuple(range(len(in_names), n_io)), keep_unused=True,
    )
    sharding = NamedSharding(mesh, spec)
    ex = (sharded, in_names, out_names, out_avals, sharding)
    _program_cache[key] = ex
    return ex


def _run_device(args_dev, rt, ct):
    import jax

    sharded, in_names, out_names, out_avals, sharding = _get_executor(rt, ct)
    # the kernel fully overwrites all outputs; donate last call's buffers
    prev = _program_cache.get(("outs", rt, ct))
    if prev is None:
        prev = tuple(jax.device_put(
            np.zeros((B * av.shape[0], *av.shape[1:]), av.dtype), sharding)
            for av in out_avals)
    outs_dev = sharded(*args_dev, *prev)
    _program_cache[("outs", rt, ct)] = outs_dev
    return {n: np.array(a).reshape(B, *out_avals[i].shape)
            for i, (n, a) in enumerate(zip(out_names, outs_dev))}


def kernel(distance_matrix, max_dist, time_window_compatibility, depot,
           num_neighbors_encoder):
    import jax

    distance_matrix = np.asarray(distance_matrix, dtype=np.float32)
    time_window_compatibility = np.asarray(time_window_compatibility,
                                           dtype=np.int32)
    depot = np.asarray(depot, dtype=np.int32)
    max_dist = np.asarray(max_dist, dtype=np.float32).reshape(B)
    assert int(np.asarray(num_neighbors_encoder)) == K
    assert distance_matrix.shape == (B, N, N)

    nd_counts = [(depot[b] == 0).sum() for b in range(B)]
    max_nd = int(max(nd_counts))
    ct = max(1, -(-max_nd // P))   # stored-column tiles (must cover nd cols)
    rt = ct
    if rt > 1 and max_nd - (rt - 1) * P <= 32:
        rt -= 1                    # leftover rows are cheaper on the host
    R_dev = rt * P                 # device-processed rows per core
    C = min(ct * P, N)
    REST = N - C
    K_ACT = max(0, rt - 3)         # must match build_program

    not_eye = ~np.eye(N, dtype=bool)
    preps = [_prep_core(distance_matrix[b], time_window_compatibility[b],
                        depot[b], rt, not_eye) for b in range(B)]
    sharded, in_names, out_names, out_avals, sharding = _get_executor(rt, ct)
    assert in_names == ["x"], in_names
    concat_x = np.concatenate([p[0] for p in preps], axis=0)
    args_dev = [jax.device_put(concat_x, sharding)]

    rng = np.random.default_rng(0)
    for attempt in range(3):
        by_name = _run_device(args_dev, rt, ct)
        raw = by_name["mask"]      # [B, R_dev, C] uint8: 1 sel, 0/255 unsel
        stats = by_name["stats"]   # [B, P, 3*rt+1]
        # rest-col count: ACT-offloaded tiles ship #sel - #unsel, DVE tiles
        # ship the count directly
        cnt_rest = stats[:, :, :rt].copy()
        if REST and K_ACT:
            cnt_rest[:, :, :K_ACT] += np.float32(REST)
            cnt_rest[:, :, :K_ACT] *= np.float32(0.5)
        acc = stats[:, :, rt : 2 * rt].copy()  # #sel - #unsel over stored C
        acc[:, :, rt - 1] += stats[:, :, 3 * rt]  # split last tile
        t16 = np.float32(EPS) - stats[:, :, 2 * rt : 3 * rt]  # ~t16 (+-1 ulp)

        out = np.zeros((B, N, N), np.float32)
        ar = np.arange(N)
        for b in range(B):
            _, nd, colperm = preps[b]
            RV = min(len(nd), R_dev)
            full = np.zeros((len(nd), N), np.float32)
            full[:RV, colperm[:C]] = (raw[b, :RV, :] == 1)
            out[b, nd] = full
            dep_mask = depot[b] == 1
            out[b, dep_mask, :] = 1.0
            out[b, :, dep_mask] = 1.0
            out[b, ar, ar] = 1.0

            # exact repair of rows whose t16 is unreliable: count != 16
            # (bf16 tie at the 16/17 boundary, fold collision, or chunk
            # coverage miss all push the count off 16), < 16 eligible
            # neighbors (t16 = -3 sentinel), or |t16| below the eps guard.
            # Rows beyond the device's R_dev are computed here directly.
            count = (acc[b] + np.float32(C)) * np.float32(0.5) + cnt_rest[b]
            rr = np.arange(RV)
            pp, tt = rr % P, rr // P
            bad = ((count[pp, tt] != np.float32(K))
                   | (t16[b][pp, tt] <= -1.5)
                   | (np.abs(t16[b][pp, tt]) < 1e-3))
            for r in list(np.flatnonzero(bad)) + list(range(RV, len(nd))):
                i = int(nd[r])
                out[b, i] = _repair_row(
                    distance_matrix[b, i], time_window_compatibility[b, i],
                    depot[b], max_dist[b], i,
                )

        # audit: recompute a random sample of rows exactly on host; any
        # mismatch indicates a transient device glitch -> rerun the call
        ok = True
        for _ in range(192):
            b = int(rng.integers(B))
            i = int(rng.integers(N))
            exp = _repair_row(
                distance_matrix[b, i], time_window_compatibility[b, i],
                depot[b], max_dist[b], i,
            )
            if not np.array_equal(out[b, i], exp):
                ok = False
                break
        if ok:
            return out
    return out


# revision 28
# speedup vs baseline: 4.1902x; 1.0283x over previous
"""Trainium2 Bass kernel for nn_Actor_56916906607124 (compute_encoder_mask).

Computation (per batch instance b, row i):
  mask[b,i,j] = 1 iff  (j is among the 16 nearest time-window-compatible,
                        non-diagonal neighbors of i)  OR depot[b,i]  OR
                        depot[b,j]  OR i == j.

Sharding: pure data parallelism -- batch B=8 across 8 NeuronCores, one
instance per core.  No collectives.

Key structural facts exploited:
  * depot rows are all-ones and depot columns are all-ones in the output,
    independent of the KNN result.  Only non-depot rows (~1024 of 2048 per
    instance) need the device; the host memsets the rest while unsharding.
  * the selection key x = (twc && !diag) ? -d : -3 folds both inputs into a
    single bf16 tensor: eligible j have x = -d in (-1, 0], blocked j sit at
    -3, and the 16 nearest eligible neighbors are exactly the top-16 of x.
    bf16 rounding is monotone, so the bf16 top-16 equals the f32 top-16
    unless two values collide at the 16/17 boundary -- which the count
    check flags for exact host repair.

Per-core device program (R=1152 padded non-depot rows, 9 tiles of 128):
  DMA   : x tile [128,2048] bf16 in (4096 B/row descriptors, full rate).
  DVE   : folded = max(x[:, :1024], x[:, 1024:])  (bf16 2x mode; the Pool
          engine cannot run ALU ops on core v3);
          4x max8 over 256-wide chunks of folded -> 32 candidates;
          max8 -> top-8, match_replace, max8 -> ranks 9..16 => t16;
          bias = -t16 + eps;  is_ge count over the 896 non-stored cols
          (4x DVE mode: all-bf16 packed operands).
  ACT   : Sign(x + bias) SBUF->SBUF straight to uint8 over the 1152 stored
          cols (negatives wrap to 255; host maps ==1) with the accumulator
          shipping #sel - #unsel per row.
  DMA   : mask tile [128,1152] uint8 out on the scalar queue.

Host flags rows with count != 16 (boundary tie in bf16, fold collision, or
chunk-coverage miss -- any wrong t16 shifts the count off 16), t16 <= -2
(fewer than 16 eligible) or |t16| < 1e-3 (eps-guard margin), and recomputes
exactly those rows in f32 numpy.  ~950 of ~8100 rows on the seed-0 data;
verified to cover every differing row.
"""

from contextlib import ExitStack

import numpy as np

import concourse.bass as bass
import concourse.mybir as mybir
from concourse import bacc, tile

B, N, P = 8, 2048, 128
K = 16
EPS = 1e-7
f32 = mybir.dt.float32
bf16 = mybir.dt.bfloat16
u8 = mybir.dt.uint8
Alu = mybir.AluOpType
Act = mybir.ActivationFunctionType

_program_cache = {}


def build_program(rt=8, ct=9):
    """Device program for RT row-tiles of 128 non-depot rows; CT*128 stored
    (non-depot-first) columns."""
    key = ("nc", rt, ct)
    if key in _program_cache:
        return _program_cache[key]
    R = rt * P          # processed non-depot rows (leftover rows -> host)
    C = min(ct * P, N)  # stored (non-depot-first) columns
    REST = N - C        # trailing depot columns: counted, not stored
    K_ACT = max(0, rt - 3)  # tiles whose rest-count runs on ACT, not DVE

    nc = bacc.Bacc()
    x_h = nc.declare_dram_parameter("x", [R, N], bf16, isOutput=False)
    mask_h = nc.declare_dram_parameter("mask", [R, C], u8, isOutput=True)
    # stats columns (last tile's slots packed at the end so the bulk ships
    # before the drain): [0:rt-1] = rest-count tiles 0..rt-2,
    # [rt-1:2rt-2] = stored acc tiles 0..rt-2, [2rt-2:3rt-2] = ACT bias all
    # tiles (host recovers t16 ~ EPS - bias), [3rt-2] = rest-count last,
    # [3rt-1] = acc last, [3rt] = second-half acc of the split last tile
    stats_h = nc.declare_dram_parameter("stats", [P, 3 * rt + 1], f32,
                                        isOutput=True)

    def cnt_slot(r):
        return r if r < rt - 1 else 3 * rt - 2

    def acc_slot(r):
        return rt - 1 + r if r < rt - 1 else 3 * rt - 1

    def bias_slot(r):
        return 2 * rt - 2 + r

    H = N // 2
    with ExitStack() as ctx:
        tc = ctx.enter_context(tile.TileContext(nc))
        const = ctx.enter_context(tc.tile_pool(name="const", bufs=1))
        inp = ctx.enter_context(tc.tile_pool(name="inp", bufs=5))
        fold = ctx.enter_context(tc.tile_pool(name="fold", bufs=3))
        outp = ctx.enter_context(tc.tile_pool(name="outp", bufs=rt))
        small = ctx.enter_context(tc.tile_pool(name="small", bufs=4))
        junk = ctx.enter_context(tc.tile_pool(name="junk", bufs=2))

        v8ball = const.tile([P, 8 * rt], f32)
        stats_s = const.tile([P, 3 * rt + 1], f32)
        if REST and K_ACT:
            # ACT-offloaded tiles count the rest cols inside one full-width
            # Sign; their cnt slots are never written -- zero them so the
            # stats DMA does not ship uninitialized SBUF
            nc.gpsimd.memset(stats_s[:, 0 : min(K_ACT, rt - 1)], 0.0)
            if K_ACT == rt:
                nc.gpsimd.memset(
                    stats_s[:, 3 * rt - 2 : 3 * rt - 1], 0.0)

        pending_stores = []
        for r in range(rt):
            rows = slice(r * P, (r + 1) * P)
            x_t = inp.tile([P, N], bf16, tag="x")
            f_t = fold.tile([P, H], bf16, tag="f")
            if r == 0:
                # ramp: tile 0 loads in column pieces spread across both
                # HWDGE queues so the configs overlap, and fold1 runs in
                # halves so the DVE starts after the first two pieces land
                nc.sync.dma_start(x_t[:, 0:512], x_h[rows, 0:512])
                nc.scalar.dma_start(x_t[:, H : H + 512], x_h[rows, H : H + 512])
                nc.sync.dma_start(x_t[:, 512:H], x_h[rows, 512:H])
                nc.scalar.dma_start(x_t[:, H + 512 :], x_h[rows, H + 512 :])
                nc.vector.tensor_tensor(
                    f_t[:, 0:512], x_t[:, 0:512], x_t[:, H : H + 512], Alu.max)
                nc.vector.tensor_tensor(
                    f_t[:, 512:], x_t[:, 512:H], x_t[:, H + 512 :], Alu.max)
            else:
                nc.sync.dma_start(x_t[:], x_h[rows, :])
                # fold1[j] = max(x[j], x[j+1024]): any top-16 member of x
                # survives folding unless its partner also is one (fold
                # collision) -- then t16 comes out low and the count flags.
                nc.vector.tensor_tensor(
                    f_t[:], x_t[:, :H], x_t[:, H:], Alu.max)
            # fold2 (in place): slot j covers {j, j+512, j+1024, j+1536}
            nc.vector.tensor_tensor(
                f_t[:, 0:512], f_t[:, 0:512], f_t[:, 512:], Alu.max)
            # per-chunk top-8 of the 512 fold2 slots -> 32 candidates
            cand = small.tile([P, 32], f32, tag="cand")
            for c in range(4):
                nc.vector.max(cand[:, c * 8 : (c + 1) * 8],
                              f_t[:, c * 128 : (c + 1) * 128])
            v8a = small.tile([P, 8], f32, tag="v8a")
            nc.vector.max(v8a[:], cand[:])
            cand2 = small.tile([P, 32], f32, tag="cand2")
            nc.vector.match_replace(cand2[:], v8a[:], cand[:], -1e30)
            v8b = v8ball[:, r * 8 : (r + 1) * 8]
            nc.vector.max(v8b, cand2[:])
            t16 = v8ball[:, r * 8 + 7 : r * 8 + 8]
            # ACT bias: -t16 + EPS (EPS < any bf16 gap at |t16| >= 1e-3, so
            # Sign(x + bias) > 0  <=>  x >= t16; |t16| < 1e-3 rows flagged)
            bias = stats_s[:, bias_slot(r) : bias_slot(r) + 1]
            nc.vector.tensor_scalar(bias, t16, -1.0, EPS, Alu.mult, Alu.add)
            # stored mask: Sign gives 1 / 0 / -1(->255 as uint8); the
            # accumulator ships  #sel - #unsel  so count = (acc + width) / 2.
            # ACT-offloaded tiles Sign the FULL row in one pass (the [C:]
            # region is junk for the store but its accum IS the rest count);
            # the last tile runs in halves so its store drains while the
            # second half is still on the ACT engine.
            acc_ap = stats_s[:, acc_slot(r) : acc_slot(r) + 1]
            if REST and r < K_ACT:
                out_t = outp.tile([P, N], u8, tag="outw")
                nc.scalar.activation(out_t[:], x_t[:], Act.Sign, bias=bias,
                                     accum_out=acc_ap)
                pending_stores.append((rows, out_t, None))
            elif r == rt - 1:
                out_t = outp.tile([P, C], u8, tag="out")
                hc = (C // 2 + 127) & ~127
                nc.scalar.activation(out_t[:, :hc], x_t[:, :hc], Act.Sign,
                                     bias=bias, accum_out=acc_ap)
                nc.scalar.activation(out_t[:, hc:], x_t[:, hc:C], Act.Sign,
                                     bias=bias,
                                     accum_out=stats_s[:, 3 * rt : 3 * rt + 1])
                pending_stores.append((rows, out_t, hc))
            else:
                out_t = outp.tile([P, C], u8, tag="out")
                nc.scalar.activation(out_t[:], x_t[:, :C], Act.Sign, bias=bias,
                                     accum_out=acc_ap)
                pending_stores.append((rows, out_t, None))
            if not REST:
                nc.vector.memset(stats_s[:, cnt_slot(r) : cnt_slot(r) + 1], 0.0)
            elif r >= K_ACT:
                # last tiles keep the count on DVE so the ACT tail is short;
                # in-place over x so no extra SBUF, 4x DVE mode (all bf16)
                nc.vector.tensor_scalar(
                    x_t[:, C:], x_t[:, C:], t16, None, Alu.is_ge, Alu.add,
                    accum_out=stats_s[:, cnt_slot(r) : cnt_slot(r) + 1])

        # mask stores ride the sync queue AFTER every load: each store's
        # dependency wait blocks the issuing sequencer, so putting them on
        # the scalar queue would stall the next tile's Sign dispatch
        for rows, out_t, hc in pending_stores:
            if hc is None:
                nc.sync.dma_start(mask_h[rows, :], out_t[:, :C])
            else:
                nc.sync.dma_start(mask_h[rows, :hc], out_t[:, :hc])
                nc.sync.dma_start(mask_h[rows, hc:], out_t[:, hc:])
        # stats ship in two pieces: the bulk (everything but the last tile's
        # slots, which sit contiguously at the end) leaves as soon as tile
        # rt-2 finishes; only 3 tail columns ride the drain path
        nc.scalar.dma_start(stats_h[:, : 3 * rt - 2],
                            stats_s[:, : 3 * rt - 2])
        nc.scalar.dma_start(stats_h[:, 3 * rt - 2 :],
                            stats_s[:, 3 * rt - 2 :])

    nc.compile()
    _program_cache[key] = nc
    return nc


def _repair_row(d_row, twc_row, depot_b, max_dist_b, i):
    """Exact float32 re-computation of reference row i (handles ties)."""
    n = d_row.shape[0]
    m = (twc_row == 0).astype(np.float32)
    m[i] = np.float32(1.0)
    big = (m * np.float32(max_dist_b)) * np.float32(10.0)
    dist = d_row * (np.float32(1.0) - m) + big
    idx = np.argsort(dist, kind="stable")[:K]
    knn = np.zeros(n, np.float32)
    knn[idx] = 1.0
    knn *= (twc_row == 1)
    dep = (depot_b + depot_b[i]) > 0
    out = ((knn > 0) | dep | (np.arange(n) == i)).astype(np.float32)
    return out


def _prep_core(d_b, twc_b, depot_b, rt, not_eye):
    """Build the per-core compacted selection-key tensor + index maps."""
    R = rt * P
    bf = mybir.dt.np(bf16)
    nd = np.flatnonzero(depot_b == 0)
    dep = np.flatnonzero(depot_b == 1)
    colperm = np.concatenate([nd, dep])
    xf = np.where((twc_b == 1) & not_eye, -d_b, np.float32(-3.0))
    xc = np.full((R, N), np.float32(-3.0), np.float32)
    nv = min(len(nd), R)
    xc[:nv] = xf[nd[:nv]][:, colperm]
    return xc.astype(bf), nd, colperm


def _get_executor(rt=8, ct=9):
    """Build the 8-core shard_map executable once (mirrors
    bass2jax.run_bass_via_pjrt, but cached so repeat calls skip retracing)."""
    key = ("exec", rt, ct)
    if key in _program_cache:
        return _program_cache[key]
    import jax
    from jax.sharding import Mesh, NamedSharding, PartitionSpec
    from jax.experimental.shard_map import shard_map
    from concourse import bass2jax
    from concourse.bass2jax import _bass_exec_p, install_neuronx_cc_hook

    nc = build_program(rt, ct)
    install_neuronx_cc_hook()
    partition_name = (nc.partition_id_tensor.name
                      if nc.partition_id_tensor else None)
    in_names, out_names, out_avals = [], [], []
    for alloc in nc.m.functions[0].allocations:
        if not isinstance(alloc, mybir.MemoryLocationSet):
            continue
        name = alloc.memorylocations[0].name
        if alloc.kind == "ExternalInput":
            if name != partition_name:
                in_names.append(name)
        elif alloc.kind == "ExternalOutput":
            out_names.append(name)
            out_avals.append(jax.core.ShapedArray(
                tuple(alloc.tensor_shape), mybir.dt.np(alloc.dtype)))
    all_in_names = list(in_names) + list(out_names)
    if partition_name is not None:
        all_in_names.append(partition_name)

    def _body(*args):
        operands = list(args)
        if partition_name is not None:
            operands.append(bass2jax.partition_id_tensor())
        return tuple(_bass_exec_p.bind(
            *operands,
            out_avals=tuple(out_avals),
            in_names=tuple(all_in_names),
            out_names=tuple(out_names),
            lowering_input_output_aliases=(),
            sim_require_finite=True,
            sim_require_nnan=True,
            nc=nc,
        ))

    devices = jax.devices()[:B]
    mesh = Mesh(np.asarray(devices), ("core",))
    spec = PartitionSpec("core")
    n_io = len(in_names) + len(out_names)
    sharded = jax.jit(
        shard_map(_body, mesh=mesh, in_specs=(spec,) * n_io,
                  out_specs=(spec,) * len(out_names), check_rep=False),
        donate_argnums=tuple(range(len(in_names), n_io)), keep_unused=True,
    )
    sharding = NamedSharding(mesh, spec)
    ex = (sharded, in_names, out_names, out_avals, sharding)
    _program_cache[key] = ex
    return ex


def _run_device(args_dev, rt, ct):
    import jax

    sharded, in_names, out_names, out_avals, sharding = _get_executor(rt, ct)
    # the kernel fully overwrites all outputs; donate last call's buffers
    prev = _program_cache.get(("outs", rt, ct))
    if prev is None:
        prev = tuple(jax.device_put(
            np.zeros((B * av.shape[0], *av.shape[1:]), av.dtype), sharding)
            for av in out_avals)
    outs_dev = sharded(*args_dev, *prev)
    _program_cache[("outs", rt, ct)] = outs_dev
    return {n: np.array(a).reshape(B, *out_avals[i].shape)
            for i, (n, a) in enumerate(zip(out_names, outs_dev))}


def kernel(distance_matrix, max_dist, time_window_compatibility, depot,
           num_neighbors_encoder):
    import jax

    distance_matrix = np.asarray(distance_matrix, dtype=np.float32)
    time_window_compatibility = np.asarray(time_window_compatibility,
                                           dtype=np.int32)
    depot = np.asarray(depot, dtype=np.int32)
    max_dist = np.asarray(max_dist, dtype=np.float32).reshape(B)
    assert int(np.asarray(num_neighbors_encoder)) == K
    assert distance_matrix.shape == (B, N, N)

    nd_counts = [(depot[b] == 0).sum() for b in range(B)]
    max_nd = int(max(nd_counts))
    ct = max(1, -(-max_nd // P))   # stored-column tiles (must cover nd cols)
    rt = ct
    if rt > 1 and max_nd - (rt - 1) * P <= 32:
        rt -= 1                    # leftover rows are cheaper on the host
    R_dev = rt * P                 # device-processed rows per core
    C = min(ct * P, N)
    REST = N - C
    K_ACT = max(0, rt - 3)         # must match build_program

    not_eye = ~np.eye(N, dtype=bool)
    preps = [_prep_core(distance_matrix[b], time_window_compatibility[b],
                        depot[b], rt, not_eye) for b in range(B)]
    sharded, in_names, out_names, out_avals, sharding = _get_executor(rt, ct)
    assert in_names == ["x"], in_names
    concat_x = np.concatenate([p[0] for p in preps], axis=0)
    args_dev = [jax.device_put(concat_x, sharding)]

    rng = np.random.default_rng(0)
    for attempt in range(3):
        by_name = _run_device(args_dev, rt, ct)
        raw = by_name["mask"]      # [B, R_dev, C] uint8: 1 sel, 0/255 unsel
        stats = by_name["stats"]   # [B, P, 3*rt+1]; layout per build_program
        cnt_rest = np.concatenate(
            [stats[:, :, : rt - 1], stats[:, :, 3 * rt - 2 : 3 * rt - 1]], -1)
        acc = np.concatenate(
            [stats[:, :, rt - 1 : 2 * rt - 2],
             stats[:, :, 3 * rt - 1 : 3 * rt]], -1).copy()
        acc[:, :, rt - 1] += stats[:, :, 3 * rt]  # split last tile
        t16 = np.float32(EPS) - stats[:, :, 2 * rt - 2 : 3 * rt - 2]
        # count over the whole row: ACT-offloaded tiles folded the rest cols
        # into one full-width accum (base N); DVE tiles ship the rest count
        # directly (base C)
        base = np.where((np.arange(rt) < K_ACT) & (REST > 0),
                        np.float32(N), np.float32(C))
        count_all = (acc + base) * np.float32(0.5) + cnt_rest

        out = np.zeros((B, N, N), np.float32)
        ar = np.arange(N)
        for b in range(B):
            _, nd, colperm = preps[b]
            RV = min(len(nd), R_dev)
            full = np.zeros((len(nd), N), np.float32)
            full[:RV, colperm[:C]] = (raw[b, :RV, :] == 1)
            out[b, nd] = full
            dep_mask = depot[b] == 1
            out[b, dep_mask, :] = 1.0
            out[b, :, dep_mask] = 1.0
            out[b, ar, ar] = 1.0

            # exact repair of rows whose t16 is unreliable: count != 16
            # (bf16 tie at the 16/17 boundary, fold collision, or chunk
            # coverage miss all push the count off 16), < 16 eligible
            # neighbors (t16 = -3 sentinel), or |t16| below the eps guard.
            # Rows beyond the device's R_dev are computed here directly.
            count = count_all[b]
            rr = np.arange(RV)
            pp, tt = rr % P, rr // P
            bad = ((count[pp, tt] != np.float32(K))
                   | (t16[b][pp, tt] <= -1.5)
                   | (np.abs(t16[b][pp, tt]) < 1e-3))
            for r in list(np.flatnonzero(bad)) + list(range(RV, len(nd))):
                i = int(nd[r])
                out[b, i] = _repair_row(
                    distance_matrix[b, i], time_window_compatibility[b, i],
                    depot[b], max_dist[b], i,
                )

        # audit: recompute a random sample of rows exactly on host; any
        # mismatch indicates a transient device glitch -> rerun the call
        ok = True
        for _ in range(192):
            b = int(rng.integers(B))
            i = int(rng.integers(N))
            exp = _repair_row(
                distance_matrix[b, i], time_window_compatibility[b, i],
                depot[b], max_dist[b], i,
            )
            if not np.array_equal(out[b, i], exp):
                ok = False
                break
        if ok:
            return out
    return out


# revision 33
# speedup vs baseline: 4.2680x; 1.0186x over previous
"""Trainium2 Bass kernel for nn_Actor_56916906607124 (compute_encoder_mask).

Computation (per batch instance b, row i):
  mask[b,i,j] = 1 iff  (j is among the 16 nearest time-window-compatible,
                        non-diagonal neighbors of i)  OR depot[b,i]  OR
                        depot[b,j]  OR i == j.

Sharding: pure data parallelism -- batch B=8 across 8 NeuronCores, one
instance per core.  No collectives.

Key structural facts exploited:
  * depot rows are all-ones and depot columns are all-ones in the output,
    independent of the KNN result.  Only non-depot rows (~1024 of 2048 per
    instance) need the device; the host memsets the rest while unsharding.
  * the selection key x = (twc && !diag) ? -d : -3 folds both inputs into a
    single bf16 tensor: eligible j have x = -d in (-1, 0], blocked j sit at
    -3, and the 16 nearest eligible neighbors are exactly the top-16 of x.
    bf16 rounding is monotone, so the bf16 top-16 equals the f32 top-16
    unless two values collide at the 16/17 boundary -- which the count
    check flags for exact host repair.

Per-core device program (R=1152 padded non-depot rows, 9 tiles of 128):
  DMA   : x tile [128,2048] bf16 in (4096 B/row descriptors, full rate).
  DVE   : folded = max(x[:, :1024], x[:, 1024:])  (bf16 2x mode; the Pool
          engine cannot run ALU ops on core v3);
          4x max8 over 256-wide chunks of folded -> 32 candidates;
          max8 -> top-8, match_replace, max8 -> ranks 9..16 => t16;
          bias = -t16 + eps;  is_ge count over the 896 non-stored cols
          (4x DVE mode: all-bf16 packed operands).
  ACT   : Sign(x + bias) SBUF->SBUF straight to uint8 over the 1152 stored
          cols (negatives wrap to 255; host maps ==1) with the accumulator
          shipping #sel - #unsel per row.
  DMA   : mask tile [128,1152] uint8 out on the scalar queue.

Host flags rows with count != 16 (boundary tie in bf16, fold collision, or
chunk-coverage miss -- any wrong t16 shifts the count off 16), t16 <= -2
(fewer than 16 eligible) or |t16| < 1e-3 (eps-guard margin), and recomputes
exactly those rows in f32 numpy.  ~950 of ~8100 rows on the seed-0 data;
verified to cover every differing row.
"""

from contextlib import ExitStack

import numpy as np

import concourse.bass as bass
import concourse.mybir as mybir
from concourse import bacc, tile

B, N, P = 8, 2048, 128
K = 16
EPS = 1e-7
f32 = mybir.dt.float32
bf16 = mybir.dt.bfloat16
u8 = mybir.dt.uint8
Alu = mybir.AluOpType
Act = mybir.ActivationFunctionType

_program_cache = {}


def build_program(rt=8, ct=9):
    """Device program for RT row-tiles of 128 non-depot rows; CT*128 stored
    (non-depot-first) columns."""
    key = ("nc", rt, ct)
    if key in _program_cache:
        return _program_cache[key]
    R = rt * P          # processed non-depot rows (leftover rows -> host)
    C = min(ct * P, N)  # stored (non-depot-first) columns
    REST = N - C        # trailing depot columns: counted, not stored
    K_ACT = max(0, rt - 3)  # tiles whose rest-count runs on ACT, not DVE

    nc = bacc.Bacc()
    x_h = nc.declare_dram_parameter("x", [R, N], bf16, isOutput=False)
    mask_h = nc.declare_dram_parameter("mask", [R, C], u8, isOutput=True)
    # last tile's mask is produced on DVE as bf16 0/1 (is_ge in 4x mode)
    # so the drain does not wait for the ACT engine
    maskl_h = nc.declare_dram_parameter("maskl", [P, C], bf16, isOutput=True)
    # stats columns (last tile's slots packed at the end so the bulk ships
    # before the drain): [0:rt-1] = rest-count tiles 0..rt-2,
    # [rt-1:2rt-2] = stored acc tiles 0..rt-2, [2rt-2:3rt-2] = ACT bias all
    # tiles (host recovers t16 ~ EPS - bias), [3rt-2] = rest-count last,
    # [3rt-1] = acc last, [3rt] = second-half acc of the split last tile
    stats_h = nc.declare_dram_parameter("stats", [P, 3 * rt + 1], f32,
                                        isOutput=True)

    def cnt_slot(r):
        return r if r < rt - 1 else 3 * rt - 2

    def acc_slot(r):
        return rt - 1 + r if r < rt - 1 else 3 * rt - 1

    def bias_slot(r):
        return 2 * rt - 2 + r

    H = N // 2
    with ExitStack() as ctx:
        tc = ctx.enter_context(tile.TileContext(nc))
        const = ctx.enter_context(tc.tile_pool(name="const", bufs=1))
        inp = ctx.enter_context(tc.tile_pool(name="inp", bufs=5))
        fold = ctx.enter_context(tc.tile_pool(name="fold", bufs=3))
        outp = ctx.enter_context(tc.tile_pool(name="outp", bufs=rt))
        small = ctx.enter_context(tc.tile_pool(name="small", bufs=4))
        junk = ctx.enter_context(tc.tile_pool(name="junk", bufs=2))

        v8ball = const.tile([P, 8 * rt], f32)
        stats_s = const.tile([P, 3 * rt + 1], f32)
        if REST and K_ACT:
            # ACT-offloaded tiles count the rest cols inside one full-width
            # Sign; their cnt slots are never written -- zero them so the
            # stats DMA does not ship uninitialized SBUF
            nc.gpsimd.memset(stats_s[:, 0 : min(K_ACT, rt - 1)], 0.0)
            if K_ACT == rt:
                nc.gpsimd.memset(
                    stats_s[:, 3 * rt - 2 : 3 * rt - 1], 0.0)

        pending_stores = []
        for r in range(rt):
            rows = slice(r * P, (r + 1) * P)
            x_t = inp.tile([P, N], bf16, tag="x")
            f_t = fold.tile([P, H], bf16, tag="f")
            if r == 0:
                # ramp: tile 0 loads in column pieces spread across both
                # HWDGE queues so the configs overlap, and fold1 runs in
                # halves so the DVE starts after the first two pieces land
                nc.sync.dma_start(x_t[:, 0:512], x_h[rows, 0:512])
                nc.scalar.dma_start(x_t[:, H : H + 512], x_h[rows, H : H + 512])
                nc.sync.dma_start(x_t[:, 512:H], x_h[rows, 512:H])
                nc.scalar.dma_start(x_t[:, H + 512 :], x_h[rows, H + 512 :])
                nc.vector.tensor_tensor(
                    f_t[:, 0:512], x_t[:, 0:512], x_t[:, H : H + 512], Alu.max)
                nc.vector.tensor_tensor(
                    f_t[:, 512:], x_t[:, 512:H], x_t[:, H + 512 :], Alu.max)
            else:
                nc.sync.dma_start(x_t[:], x_h[rows, :])
                # fold1[j] = max(x[j], x[j+1024]): any top-16 member of x
                # survives folding unless its partner also is one (fold
                # collision) -- then t16 comes out low and the count flags.
                nc.vector.tensor_tensor(
                    f_t[:], x_t[:, :H], x_t[:, H:], Alu.max)
            # fold2 (in place): slot j covers {j, j+512, j+1024, j+1536}
            nc.vector.tensor_tensor(
                f_t[:, 0:512], f_t[:, 0:512], f_t[:, 512:], Alu.max)
            # per-chunk top-8 of the 512 fold2 slots -> 32 candidates
            cand = small.tile([P, 32], f32, tag="cand")
            for c in range(4):
                nc.vector.max(cand[:, c * 8 : (c + 1) * 8],
                              f_t[:, c * 128 : (c + 1) * 128])
            v8a = small.tile([P, 8], f32, tag="v8a")
            nc.vector.max(v8a[:], cand[:])
            cand2 = small.tile([P, 32], f32, tag="cand2")
            nc.vector.match_replace(cand2[:], v8a[:], cand[:], -1e30)
            v8b = v8ball[:, r * 8 : (r + 1) * 8]
            nc.vector.max(v8b, cand2[:])
            t16 = v8ball[:, r * 8 + 7 : r * 8 + 8]
            # ACT bias: -t16 + EPS (EPS < any bf16 gap at |t16| >= 1e-3, so
            # Sign(x + bias) > 0  <=>  x >= t16; |t16| < 1e-3 rows flagged)
            bias = stats_s[:, bias_slot(r) : bias_slot(r) + 1]
            nc.vector.tensor_scalar(bias, t16, -1.0, EPS, Alu.mult, Alu.add)
            # stored mask: Sign gives 1 / 0 / -1(->255 as uint8); the
            # accumulator ships  #sel - #unsel  so count = (acc + width) / 2.
            # ACT-offloaded tiles Sign the FULL row in one pass (the [C:]
            # region is junk for the store but its accum IS the rest count);
            # the last tile runs in halves so its store drains while the
            # second half is still on the ACT engine.
            acc_ap = stats_s[:, acc_slot(r) : acc_slot(r) + 1]
            if r == rt - 1:
                # last tile: mask on DVE as bf16 0/1 (4x mode), accum is the
                # stored-col count directly; ACT plays no part in the drain
                outl_t = outp.tile([P, C], bf16, tag="outl")
                nc.vector.tensor_scalar(
                    outl_t[:], x_t[:, :C], t16, None, Alu.is_ge, Alu.add,
                    accum_out=acc_ap)
                nc.vector.memset(stats_s[:, 3 * rt : 3 * rt + 1], 0.0)
            elif REST and r < K_ACT:
                out_t = outp.tile([P, N], u8, tag="outw")
                nc.scalar.activation(out_t[:], x_t[:], Act.Sign, bias=bias,
                                     accum_out=acc_ap)
                pending_stores.append((rows, out_t))
            else:
                out_t = outp.tile([P, C], u8, tag="out")
                nc.scalar.activation(out_t[:], x_t[:, :C], Act.Sign, bias=bias,
                                     accum_out=acc_ap)
                pending_stores.append((rows, out_t))
            if not REST:
                nc.vector.memset(stats_s[:, cnt_slot(r) : cnt_slot(r) + 1], 0.0)
            elif r >= K_ACT:
                # late tiles keep the rest count on DVE (4x mode, in-place
                # over x) so the ACT stream ends earlier
                nc.vector.tensor_scalar(
                    x_t[:, C:], x_t[:, C:], t16, None, Alu.is_ge, Alu.add,
                    accum_out=stats_s[:, cnt_slot(r) : cnt_slot(r) + 1])

        # mask stores ride the sync queue AFTER every load: each store's
        # dependency wait blocks the issuing sequencer, so putting them on
        # the scalar queue would stall the next tile's Sign dispatch
        for rows, out_t in pending_stores:
            nc.sync.dma_start(mask_h[rows, :], out_t[:, :C])
        nc.sync.dma_start(maskl_h[:, :], outl_t[:])
        # stats ship in two pieces: the bulk (everything but the last tile's
        # slots, which sit contiguously at the end) leaves as soon as tile
        # rt-2 finishes; only 3 tail columns ride the drain path
        nc.scalar.dma_start(stats_h[:, : 3 * rt - 2],
                            stats_s[:, : 3 * rt - 2])
        nc.scalar.dma_start(stats_h[:, 3 * rt - 2 :],
                            stats_s[:, 3 * rt - 2 :])

    nc.compile()
    _program_cache[key] = nc
    return nc


def _repair_row(d_row, twc_row, depot_b, max_dist_b, i):
    """Exact float32 re-computation of reference row i (handles ties)."""
    n = d_row.shape[0]
    m = (twc_row == 0).astype(np.float32)
    m[i] = np.float32(1.0)
    big = (m * np.float32(max_dist_b)) * np.float32(10.0)
    dist = d_row * (np.float32(1.0) - m) + big
    idx = np.argsort(dist, kind="stable")[:K]
    knn = np.zeros(n, np.float32)
    knn[idx] = 1.0
    knn *= (twc_row == 1)
    dep = (depot_b + depot_b[i]) > 0
    out = ((knn > 0) | dep | (np.arange(n) == i)).astype(np.float32)
    return out


def _prep_core(d_b, twc_b, depot_b, rt, not_eye):
    """Build the per-core compacted selection-key tensor + index maps."""
    R = rt * P
    bf = mybir.dt.np(bf16)
    nd = np.flatnonzero(depot_b == 0)
    dep = np.flatnonzero(depot_b == 1)
    colperm = np.concatenate([nd, dep])
    xf = np.where((twc_b == 1) & not_eye, -d_b, np.float32(-3.0))
    xc = np.full((R, N), np.float32(-3.0), np.float32)
    nv = min(len(nd), R)
    xc[:nv] = xf[nd[:nv]][:, colperm]
    return xc.astype(bf), nd, colperm


def _get_executor(rt=8, ct=9):
    """Build the 8-core shard_map executable once (mirrors
    bass2jax.run_bass_via_pjrt, but cached so repeat calls skip retracing)."""
    key = ("exec", rt, ct)
    if key in _program_cache:
        return _program_cache[key]
    import jax
    from jax.sharding import Mesh, NamedSharding, PartitionSpec
    from jax.experimental.shard_map import shard_map
    from concourse import bass2jax
    from concourse.bass2jax import _bass_exec_p, install_neuronx_cc_hook

    nc = build_program(rt, ct)
    install_neuronx_cc_hook()
    partition_name = (nc.partition_id_tensor.name
                      if nc.partition_id_tensor else None)
    in_names, out_names, out_avals = [], [], []
    for alloc in nc.m.functions[0].allocations:
        if not isinstance(alloc, mybir.MemoryLocationSet):
            continue
        name = alloc.memorylocations[0].name
        if alloc.kind == "ExternalInput":
            if name != partition_name:
                in_names.append(name)
        elif alloc.kind == "ExternalOutput":
            out_names.append(name)
            out_avals.append(jax.core.ShapedArray(
                tuple(alloc.tensor_shape), mybir.dt.np(alloc.dtype)))
    all_in_names = list(in_names) + list(out_names)
    if partition_name is not None:
        all_in_names.append(partition_name)

    def _body(*args):
        operands = list(args)
        if partition_name is not None:
            operands.append(bass2jax.partition_id_tensor())
        return tuple(_bass_exec_p.bind(
            *operands,
            out_avals=tuple(out_avals),
            in_names=tuple(all_in_names),
            out_names=tuple(out_names),
            lowering_input_output_aliases=(),
            sim_require_finite=True,
            sim_require_nnan=True,
            nc=nc,
        ))

    devices = jax.devices()[:B]
    mesh = Mesh(np.asarray(devices), ("core",))
    spec = PartitionSpec("core")
    n_io = len(in_names) + len(out_names)
    sharded = jax.jit(
        shard_map(_body, mesh=mesh, in_specs=(spec,) * n_io,
                  out_specs=(spec,) * len(out_names), check_rep=False),
        donate_argnums=tuple(range(len(in_names), n_io)), keep_unused=True,
    )
    sharding = NamedSharding(mesh, spec)
    ex = (sharded, in_names, out_names, out_avals, sharding)
    _program_cache[key] = ex
    return ex


def _run_device(args_dev, rt, ct):
    import jax

    sharded, in_names, out_names, out_avals, sharding = _get_executor(rt, ct)
    # the kernel fully overwrites all outputs; donate last call's buffers
    prev = _program_cache.get(("outs", rt, ct))
    if prev is None:
        prev = tuple(jax.device_put(
            np.zeros((B * av.shape[0], *av.shape[1:]), av.dtype), sharding)
            for av in out_avals)
    outs_dev = sharded(*args_dev, *prev)
    _program_cache[("outs", rt, ct)] = outs_dev
    return {n: np.array(a).reshape(B, *out_avals[i].shape)
            for i, (n, a) in enumerate(zip(out_names, outs_dev))}


def kernel(distance_matrix, max_dist, time_window_compatibility, depot,
           num_neighbors_encoder):
    import jax

    distance_matrix = np.asarray(distance_matrix, dtype=np.float32)
    time_window_compatibility = np.asarray(time_window_compatibility,
                                           dtype=np.int32)
    depot = np.asarray(depot, dtype=np.int32)
    max_dist = np.asarray(max_dist, dtype=np.float32).reshape(B)
    assert int(np.asarray(num_neighbors_encoder)) == K
    assert distance_matrix.shape == (B, N, N)

    nd_counts = [(depot[b] == 0).sum() for b in range(B)]
    max_nd = int(max(nd_counts))
    ct = max(1, -(-max_nd // P))   # stored-column tiles (must cover nd cols)
    rt = ct
    if rt > 1 and max_nd - (rt - 1) * P <= 32:
        rt -= 1                    # leftover rows are cheaper on the host
    R_dev = rt * P                 # device-processed rows per core
    C = min(ct * P, N)
    REST = N - C
    K_ACT = max(0, rt - 3)         # must match build_program

    not_eye = ~np.eye(N, dtype=bool)
    preps = [_prep_core(distance_matrix[b], time_window_compatibility[b],
                        depot[b], rt, not_eye) for b in range(B)]
    sharded, in_names, out_names, out_avals, sharding = _get_executor(rt, ct)
    assert in_names == ["x"], in_names
    concat_x = np.concatenate([p[0] for p in preps], axis=0)
    args_dev = [jax.device_put(concat_x, sharding)]

    rng = np.random.default_rng(0)
    for attempt in range(3):
        by_name = _run_device(args_dev, rt, ct)
        raw = by_name["mask"]      # [B, R_dev, C] uint8: 1 sel, 0/255 unsel
        stats = by_name["stats"]   # [B, P, 3*rt+1]; layout per build_program
        cnt_rest = np.concatenate(
            [stats[:, :, : rt - 1], stats[:, :, 3 * rt - 2 : 3 * rt - 1]], -1)
        acc = np.concatenate(
            [stats[:, :, rt - 1 : 2 * rt - 2],
             stats[:, :, 3 * rt - 1 : 3 * rt]], -1).copy()
        acc[:, :, rt - 1] += stats[:, :, 3 * rt]  # split last tile
        t16 = np.float32(EPS) - stats[:, :, 2 * rt - 2 : 3 * rt - 2]
        # count over the whole row: ACT-offloaded tiles folded the rest cols
        # into one full-width accum (base N); DVE tiles ship the rest count
        # directly (base C)
        base = np.where((np.arange(rt) < K_ACT) & (REST > 0),
                        np.float32(N), np.float32(C))
        count_all = (acc + base) * np.float32(0.5) + cnt_rest
        # last tile's acc slot is a direct is_ge count, not #sel - #unsel
        count_all[:, :, rt - 1] = acc[:, :, rt - 1] + cnt_rest[:, :, rt - 1]

        out = np.zeros((B, N, N), np.float32)
        ar = np.arange(N)
        for b in range(B):
            _, nd, colperm = preps[b]
            RV = min(len(nd), R_dev)
            sel = (raw[b] == 1)
            sel[(rt - 1) * P :] = (by_name["maskl"][b] == 1.0)
            full = np.zeros((len(nd), N), np.float32)
            full[:RV, colperm[:C]] = sel[:RV]
            out[b, nd] = full
            dep_mask = depot[b] == 1
            out[b, dep_mask, :] = 1.0
            out[b, :, dep_mask] = 1.0
            out[b, ar, ar] = 1.0

            # exact repair of rows whose t16 is unreliable: count != 16
            # (bf16 tie at the 16/17 boundary, fold collision, or chunk
            # coverage miss all push the count off 16), < 16 eligible
            # neighbors (t16 = -3 sentinel), or |t16| below the eps guard.
            # Rows beyond the device's R_dev are computed here directly.
            count = count_all[b]
            rr = np.arange(RV)
            pp, tt = rr % P, rr // P
            bad = ((count[pp, tt] != np.float32(K))
                   | (t16[b][pp, tt] <= -1.5)
                   | (np.abs(t16[b][pp, tt]) < 1e-3))
            for r in list(np.flatnonzero(bad)) + list(range(RV, len(nd))):
                i = int(nd[r])
                out[b, i] = _repair_row(
                    distance_matrix[b, i], time_window_compatibility[b, i],
                    depot[b], max_dist[b], i,
                )

        # audit: recompute a random sample of rows exactly on host; any
        # mismatch indicates a transient device glitch -> rerun the call
        ok = True
        for _ in range(192):
            b = int(rng.integers(B))
            i = int(rng.integers(N))
            exp = _repair_row(
                distance_matrix[b, i], time_window_compatibility[b, i],
                depot[b], max_dist[b], i,
            )
            if not np.array_equal(out[b, i], exp):
                ok = False
                break
        if ok:
            return out
    return out
